# revision 1
# baseline (speedup 1.0000x reference)
"""Trainium2 Bass kernel for sliding-window Pearson correlation attention.

Input  x: [512, 2, 32768] f32.
Output attentions: [512, 32669] f32 = relu(corr - mean_b(corr)) where corr is
the per-batch sliding-window (w=100) Pearson correlation of the two channels.

Sharding: split the T/output dimension across the 8 cores (4084 output
columns each, + 99-column halo on the input). Every core sees all 512
batches, so the batch-mean is computed locally - no collective needed.

Layout: batch-major (partition = batch row, 4 tiles of 128). Windowed sums
are computed with the DVE scan instruction via the recurrence
    s[i+1] = s[i] + a[i+100] - a[i]
(one streaming pass per sequence, all 5 sequences pre-scaled by w so the
Pearson formula reduces to plain tensor-tensor ops). Squares and
rsqrt (exp(-0.5*ln)) run on ScalarE, three of the elementwise products on
GpSimd, the batch mean + partition broadcast on the PE (ones-matmuls), and
the variance subtract reads its second operand from PSUM to keep the shared
DVE/GpSimd SBUF port free.
"""

import numpy as np

import concourse.bass as bass
import concourse.mybir as mybir
import concourse.tile as tile
from concourse.bass_utils import run_bass_kernel_spmd

WIN = 100
B = 512
CH = 2
T = 32768
N = T - WIN + 1  # 32669
NCORES = 8
NLOC = 4084  # output columns per core (8*4084 = 32672 >= N; tail dropped)
FIN = NLOC + WIN - 1  # 4183 input columns per core
TPAD = (NCORES - 1) * NLOC + FIN  # 32771 (input padded with 3 zero cols)
P = 128
NBT = B // P  # 4 batch tiles
NCHUNK = 4
F = NLOC // NCHUNK  # 1021 output columns per chunk
H = F + WIN - 1  # 1120 input columns per chunk

f32 = mybir.dt.float32
AOT = mybir.ActivationFunctionType
ALU = mybir.AluOpType
AXL = mybir.AxisListType

REPEAT = 1  # bench-only: repeat the whole computation inside one NEFF


def _kernel_body(tc, out, xs):
    nc = tc.nc
    import contextlib

    ctx = contextlib.ExitStack()
    with ctx:
        const_pool = ctx.enter_context(tc.tile_pool(name="const", bufs=1))
        pool = ctx.enter_context(tc.tile_pool(name="work", bufs=3))
        corr_pool = ctx.enter_context(tc.tile_pool(name="corrp", bufs=6))
        row_pool = ctx.enter_context(tc.tile_pool(name="rows", bufs=2))
        psum_pool = ctx.enter_context(tc.tile_pool(name="psum", bufs=2, space="PSUM"))

        ones = const_pool.tile([P, 1], f32, tag="ones")
        nc.vector.memset(ones[:], 1.0)
        ones_row = const_pool.tile([1, P], f32, tag="ones_row")
        nc.vector.memset(ones_row[:], 1.0)

        NEG_INV_W = -1.0 / WIN
        NEG_INV_B = -1.0 / B

        SQW = float(np.sqrt(WIN))

        def wsum(dst2d, src2d):
            # dst[:, i] = sum(src[:, i:i+WIN]) for i in [0, F)
            # first-window sum via reduce, the rest via the DVE scan
            # recurrence s[i+1] = s[i] + a[i+w] - a[i].
            nc.vector.reduce_sum(dst2d[:, 0:1], src2d[:, 0:WIN], axis=AXL.X)
            nc.vector.tensor_tensor_scan(
                out=dst2d[:, 1:F],
                data0=src2d[:, WIN : WIN + F - 1],
                data1=src2d[:, 0 : F - 1],
                initial=dst2d[:, 0:1],
                op0=ALU.add,
                op1=ALU.subtract,
            )

        SPLIT = min(512, F)
        for c in range(NCHUNK * REPEAT):
            c = c % NCHUNK
            c0 = c * F
            psA = psum_pool.tile([1, SPLIT], f32, tag="psA", bufs=1)
            psB = (
                psum_pool.tile([1, F - SPLIT], f32, tag="psB", name="psB", bufs=1)
                if F > SPLIT
                else None
            )
            corrs = []
            for bt in range(NBT):
                b0 = bt * P
                x12 = pool.tile([P, CH, H], f32, tag="x12")
                nc.sync.dma_start(out=x12[:], in_=xs[b0 : b0 + P, :, c0 : c0 + H])
                x1 = x12[:, 0, :]
                x2 = x12[:, 1, :]

                # all quantities in w-scaled units: e = w*x^2, e12w = w*x1*x2
                e = pool.tile([P, CH, H], f32, tag="e")
                nc.scalar.activation(e[:], x12[:], AOT.Square, scale=SQW)
                x1s = pool.tile([P, H], f32, tag="x1s")
                nc.scalar.mul(x1s[:], x1, float(WIN))
                e12 = pool.tile([P, H], f32, tag="e12")
                nc.gpsimd.tensor_mul(e12[:], x1s[:], x2)

                s = pool.tile([P, CH, F], f32, tag="s")  # s1, s2
                se = pool.tile([P, CH, F], f32, tag="se")  # w*s11, w*s22
                # w*s12 scan lands in PSUM so the cov subtract reads it via
                # the PSUM port (SBUF port 1 stays free for GpSimd)
                s12 = psum_pool.tile([P, F], f32, tag="s12", bufs=1)
                wsum(s[:, 0, :], x1)
                wsum(s[:, 1, :], x2)
                wsum(se[:, 0, :], e[:, 0, :])
                wsum(se[:, 1, :], e[:, 1, :])
                wsum(s12[:], e12[:])

                # v = w*s11 - s1^2, channel-split so t needs only 2 PSUM banks
                t = psum_pool.tile([P, F], f32, tag="t", bufs=1)
                nc.scalar.activation(t[:], s[:, 0, :], AOT.Square)
                nc.vector.tensor_sub(se[:, 0, :], se[:, 0, :], t[:])
                t2 = psum_pool.tile([P, F], f32, tag="t", name="t2", bufs=1)
                nc.scalar.activation(t2[:], s[:, 1, :], AOT.Square)
                nc.vector.tensor_sub(se[:, 1, :], se[:, 1, :], t2[:])
                # cov = w*s12 - s1*s2
                t12 = pool.tile([P, F], f32, tag="t12")
                nc.gpsimd.tensor_mul(t12[:], s[:, 0, :], s[:, 1, :])
                cov = pool.tile([P, F], f32, tag="cov")
                nc.vector.tensor_sub(cov[:], s12[:], t12[:])
                # corr = cov * rsqrt(v1*v2);  rsqrt via exp(-0.5*ln)
                p = pool.tile([P, F], f32, tag="p")
                nc.gpsimd.tensor_mul(p[:], se[:, 0, :], se[:, 1, :])
                nc.scalar.activation(p[:], p[:], AOT.Ln)
                # rs lands in PSUM (shares the t banks - t is dead by now) so
                # the corr multiply reads via the PSUM port, leaving SBUF
                # port 1 free for the GpSimd products.
                rs = psum_pool.tile([P, F], f32, tag="t", name="rs", bufs=1)
                nc.scalar.activation(rs[:], p[:], AOT.Exp, scale=-0.5)
                corr = corr_pool.tile([P, F], f32, tag="corr")
                nc.vector.tensor_mul(corr[:], cov[:], rs[:])
                corrs.append(corr)

                # batch-sum via ones-matmul (accumulate over the 4 batch tiles)
                nc.tensor.matmul(
                    psA[:], ones[:], corr[:, 0:SPLIT],
                    start=(bt == 0), stop=(bt == NBT - 1),
                )
                if psB is not None:
                    nc.tensor.matmul(
                        psB[:], ones[:], corr[:, SPLIT:F],
                        start=(bt == 0), stop=(bt == NBT - 1),
                    )

            # -mean row (negate+scale while copying PSUM->SBUF)
            avg_row = row_pool.tile([1, F], f32, tag="avgrow")
            nc.scalar.mul(avg_row[:, 0:SPLIT], psA[:], NEG_INV_B)
            if psB is not None:
                nc.scalar.mul(avg_row[:, SPLIT:F], psB[:], NEG_INV_B)
            # broadcast -mean to all partitions via K=1 matmul, stage to SBUF
            avgb = psum_pool.tile([P, F], f32, tag="avgb", bufs=1)
            nc.tensor.matmul(avgb[:, 0:SPLIT], ones_row[:], avg_row[:, 0:SPLIT])
            if F > SPLIT:
                nc.tensor.matmul(avgb[:, SPLIT:F], ones_row[:], avg_row[:, SPLIT:F])
            for bt in range(NBT):
                b0 = bt * P
                corr = corrs[bt]
                nc.vector.tensor_add(corr[:], corr[:], avgb[:])
                nc.scalar.activation(corr[:], corr[:], AOT.Relu)
                nc.sync.dma_start(out=out[b0 : b0 + P, c0 : c0 + F], in_=corr[:])


def build_nc():
    from concourse import bacc

    nc = bacc.Bacc("TRN2", target_bir_lowering=False, debug=False, num_devices=NCORES)
    xs = nc.dram_tensor("xs", [B, CH, FIN], f32, kind="ExternalInput").ap()
    out = nc.dram_tensor("out", [B, NLOC], f32, kind="ExternalOutput").ap()
    with tile.TileContext(nc) as tc:
        _kernel_body(tc, out, xs)
    nc.compile()
    return nc


_NC = None


def _get_nc():
    global _NC
    if _NC is None:
        _NC = build_nc()
    return _NC


def make_in_maps(x):
    x = np.asarray(x, dtype=np.float32)
    xpad = np.zeros((B, CH, TPAD), dtype=np.float32)
    xpad[:, :, :T] = x
    return [
        {"xs": np.ascontiguousarray(xpad[:, :, c * NLOC : c * NLOC + FIN])}
        for c in range(NCORES)
    ]


def _run(x, **kwargs):
    nc = _get_nc()
    res = run_bass_kernel_spmd(nc, make_in_maps(x), core_ids=list(range(NCORES)), **kwargs)
    outs = [res.results[c]["out"] for c in range(NCORES)]
    full = np.concatenate(outs, axis=1)[:, :N].astype(np.float32)
    return full, res


def kernel(x):
    full, _ = _run(x)
    return full



# revision 3
# speedup vs baseline: 1.6562x; 1.6562x over previous
"""Trainium2 Bass kernel v5: sliding-window Pearson correlation attention.

Same math/precision scheme as v3 (bf16 pipeline, f32 ln link, DVE-only scans,
act-table preload), plus software-pipelined emission: each iteration emits the
NEXT iteration's DMA/sq/p12 before the current iteration's dependent tail, so
the in-order engine queues never sit behind a cross-engine wait with runnable
work elsewhere in the program order.

Engine split per (chunk, batch-tile) iteration (model ~8.5us):
  DVE:  5 scans, v, m12, cov, corr
  ACT:  sq, t, s1w, ln, rs (+avgb per chunk)
  Pool: p12, p, r, relu
  PE:   batch-sum matmuls (bf16, M=128 replicated)
"""

import numpy as np

import concourse.bass as bass
import concourse.mybir as mybir
import concourse.tile as tile
from concourse.bass_utils import run_bass_kernel_spmd

WIN = 100
B = 512
CH = 2
T = 32768
N = T - WIN + 1
NCORES = 8
NLOC = 4084
FIN = NLOC + WIN - 1  # 4183
TPAD = (NCORES - 1) * NLOC + FIN
P = 128
NBT = B // P  # 4
NCHUNK = 4
F = NLOC // NCHUNK  # 1021
H = F + WIN  # 1121

f32 = mybir.dt.float32
bf16 = mybir.dt.bfloat16
AOT = mybir.ActivationFunctionType
ALU = mybir.AluOpType
AXL = mybir.AxisListType

NIT = NCHUNK * NBT  # 16 iterations, chunk-major: i = c*NBT + bt


def _kernel_body(tc, out, xs):
    nc = tc.nc
    import contextlib

    ctx = contextlib.ExitStack()
    with ctx:
        const_pool = ctx.enter_context(tc.tile_pool(name="const", bufs=1))
        pool = ctx.enter_context(tc.tile_pool(name="work", bufs=3))
        xpool = ctx.enter_context(tc.tile_pool(name="xin", bufs=4))
        scan_pool = ctx.enter_context(tc.tile_pool(name="scans", bufs=5))
        corr_pool = ctx.enter_context(tc.tile_pool(name="corrp", bufs=6))
        row_pool = ctx.enter_context(tc.tile_pool(name="rows", bufs=2))
        psum_pool = ctx.enter_context(tc.tile_pool(name="psum", bufs=2, space="PSUM"))

        ones = const_pool.tile([P, P], bf16, tag="ones")
        nc.vector.memset(ones[:], 1.0)
        plnw = const_pool.tile([P, 1], f32, tag="plnw")
        nc.vector.memset(plnw[:], float(np.log(WIN)))

        SQW = float(np.sqrt(WIN))

        nc.scalar.add_instruction(
            mybir.InstLoadActFuncSet(
                name=nc.get_next_instruction_name(), act_func_set_id=6
            )
        )

        # pipeline state, indexed by iteration
        S = [dict() for _ in range(NIT)]
        prev_scans = [None] * NBT  # per-bt (slin, squad, s12t) for chunk chaining
        corrs = [None] * NBT
        ps_tiles = {}

        def stage_load(i):
            c, bt = divmod(i, NBT)
            b0 = bt * P
            x12 = xpool.tile([P, CH, H], f32, tag="x12")
            g0 = 0 if c == 0 else c * F - 1
            nc.sync.dma_start(out=x12[:], in_=xs[b0 : b0 + P, :, g0 : g0 + H])
            S[i]["x12"] = x12

        def stage_pre(i):
            # sq (ACT) + p12 (Pool) from x12
            x12 = S[i]["x12"]
            sq = pool.tile([P, CH, H], bf16, tag="sq")
            nc.scalar.activation(sq[:], x12[:], AOT.Square, scale=SQW)
            p12 = pool.tile([P, H], bf16, tag="p12")
            nc.gpsimd.tensor_mul(p12[:], x12[:, 0, :], x12[:, 1, :])
            S[i]["sq"] = sq
            S[i]["p12"] = p12

        def stage_scans(i):
            c, bt = divmod(i, NBT)
            x12, sq, p12 = S[i]["x12"], S[i]["sq"], S[i]["p12"]
            slin = scan_pool.tile([P, CH, F], bf16, tag="slin")
            squad = scan_pool.tile([P, CH, F], bf16, tag="squad")
            s12t = scan_pool.tile([P, F], bf16, tag="s12t")

            def wsum(dst2d, src2d, pv):
                if c == 0:
                    with nc.allow_low_precision(reason="f32 accum, bf16 store"):
                        nc.vector.tensor_reduce(
                            out=dst2d[:, 0:1], in_=src2d[:, 0:WIN],
                            op=ALU.add, axis=AXL.X,
                        )
                    nc.vector.tensor_tensor_scan(
                        out=dst2d[:, 1:F],
                        data0=src2d[:, WIN : WIN + F - 1],
                        data1=src2d[:, 0 : F - 1],
                        initial=dst2d[:, 0:1],
                        op0=ALU.add, op1=ALU.subtract,
                    )
                else:
                    nc.vector.tensor_tensor_scan(
                        out=dst2d[:, 0:F],
                        data0=src2d[:, WIN : WIN + F],
                        data1=src2d[:, 0:F],
                        initial=pv,
                        op0=ALU.add, op1=ALU.subtract,
                    )

            pv = prev_scans[bt]
            wsum(slin[:, 0, :], x12[:, 0, :], pv and pv[0][:, 0, F - 1 : F])
            wsum(slin[:, 1, :], x12[:, 1, :], pv and pv[0][:, 1, F - 1 : F])
            wsum(squad[:, 0, :], sq[:, 0, :], pv and pv[1][:, 0, F - 1 : F])
            wsum(squad[:, 1, :], sq[:, 1, :], pv and pv[1][:, 1, F - 1 : F])
            wsum(s12t[:], p12[:], pv and pv[2][:, F - 1 : F])
            prev_scans[bt] = (slin, squad, s12t)
            S[i]["slin"], S[i]["squad"], S[i]["s12t"] = slin, squad, s12t

        def stage_mid(i):
            slin, squad, s12t = S[i]["slin"], S[i]["squad"], S[i]["s12t"]
            t = pool.tile([P, CH, F], bf16, tag="t")
            nc.scalar.activation(t[:], slin[:], AOT.Square)
            s1w = pool.tile([P, F], bf16, tag="s1w")
            nc.scalar.mul(s1w[:], slin[:, 0, :], 1.0 / WIN)
            v = pool.tile([P, CH, F], bf16, tag="v")
            nc.vector.tensor_sub(v[:], squad[:], t[:])
            m12 = pool.tile([P, F], bf16, tag="m12")
            nc.gpsimd.tensor_mul(m12[:], s1w[:], slin[:, 1, :])
            cov = pool.tile([P, F], bf16, tag="cov")
            nc.gpsimd.tensor_sub(cov[:], s12t[:], m12[:])
            p = pool.tile([P, F], bf16, tag="p")
            nc.vector.tensor_mul(p[:], v[:, 0, :], v[:, 1, :])
            S[i]["cov"], S[i]["p"] = cov, p

        def stage_tail(i):
            c, bt = divmod(i, NBT)
            cov, p = S[i]["cov"], S[i]["p"]
            lnp = pool.tile([P, F], f32, tag="lnp")
            nc.scalar.activation(lnp[:], p[:], AOT.Ln)
            rs = pool.tile([P, F], bf16, tag="rs")
            nc.scalar.activation(rs[:], lnp[:], AOT.Exp, scale=-0.5, bias=plnw[:])
            corr = corr_pool.tile([P, F], bf16, tag="corr")
            nc.vector.tensor_mul(corr[:], cov[:], rs[:])
            corrs[bt] = corr
            if bt == 0:
                ps_tiles[c] = psum_pool.tile(
                    [P, 1024], f32, tag="ps", name=f"ps{c}"
                )
            ps = ps_tiles[c]
            nc.tensor.matmul(
                ps[:, 0:512], ones[:], corr[:, 0:512],
                start=(bt == 0), stop=(bt == NBT - 1),
            )
            nc.tensor.matmul(
                ps[:, 512:F], ones[:], corr[:, 512:F],
                start=(bt == 0), stop=(bt == NBT - 1),
            )

        def stage_finalize(c):
            c0 = c * F
            ps = ps_tiles.pop(c)
            avgb = row_pool.tile([P, F], bf16, tag="avgb")
            nc.scalar.mul(avgb[:], ps[:, 0:F], 1.0 / B)
            for bt in range(NBT):
                b0 = bt * P
                corr = corrs[bt]
                r = pool.tile([P, F], bf16, tag="r")
                nc.vector.tensor_sub(r[:], corr[:], avgb[:])
                nc.scalar.activation(r[:], r[:], AOT.Relu)
                nc.sync.dma_start(out=out[b0 : b0 + P, c0 : c0 + F], in_=r[:])

        # software-pipelined emission; tail is delayed one iteration so the
        # ACT queue orders t/s1w(i) ahead of ln/rs(i-1) (avoids head-of-line
        # blocking on the Pool p(i-1) dependency)
        stage_load(0)
        stage_pre(0)
        stage_load(1)
        for i in range(NIT):
            if i + 1 < NIT:
                stage_pre(i + 1)
            if i + 2 < NIT:
                stage_load(i + 2)
            stage_scans(i)
            stage_mid(i)
            if i >= 1:
                stage_tail(i - 1)
                c, bt = divmod(i - 1, NBT)
                if bt == NBT - 1:
                    stage_finalize(c)
        stage_tail(NIT - 1)
        stage_finalize(NCHUNK - 1)


def build_nc():
    from concourse import bacc

    nc = bacc.Bacc("TRN2", target_bir_lowering=False, debug=False, num_devices=NCORES)
    xs = nc.dram_tensor("xs", [B, CH, FIN], f32, kind="ExternalInput").ap()
    out = nc.dram_tensor("out", [B, NLOC], bf16, kind="ExternalOutput").ap()
    with tile.TileContext(nc) as tc:
        _kernel_body(tc, out, xs)
    nc.compile()
    return nc


_NC = None


def _get_nc():
    global _NC
    if _NC is None:
        _NC = build_nc()
    return _NC


def make_in_maps(x):
    x = np.asarray(x, dtype=np.float32)
    xpad = np.zeros((B, CH, TPAD), dtype=np.float32)
    xpad[:, :, :T] = x
    return [
        {"xs": np.ascontiguousarray(xpad[:, :, c * NLOC : c * NLOC + FIN])}
        for c in range(NCORES)
    ]


def _run(x, **kwargs):
    nc = _get_nc()
    res = run_bass_kernel_spmd(nc, make_in_maps(x), core_ids=list(range(NCORES)), **kwargs)
    outs = [np.asarray(res.results[c]["out"]).astype(np.float32) for c in range(NCORES)]
    full = np.concatenate(outs, axis=1)[:, :N]
    return full, res


def kernel(x):
    full, _ = _run(x)
    return full


# revision 4
# speedup vs baseline: 1.6703x; 1.0085x over previous
"""Trainium2 Bass kernel (final): sliding-window Pearson correlation attention.

Same math/precision scheme as v3 (bf16 pipeline, f32 ln link, DVE-only scans,
act-table preload), plus software-pipelined emission: each iteration emits the
NEXT iteration's DMA/sq/p12 before the current iteration's dependent tail, so
the in-order engine queues never sit behind a cross-engine wait with runnable
work elsewhere in the program order.

Engine split per (chunk, batch-tile) iteration (model ~8.5us):
  DVE:  5 scans, v, m12, cov, corr
  ACT:  sq, t, s1w, ln, rs (+avgb per chunk)
  Pool: p12, p, r, relu
  PE:   batch-sum matmuls (bf16, M=128 replicated)
"""

import numpy as np

import concourse.bass as bass
import concourse.mybir as mybir
import concourse.tile as tile
from concourse.bass_utils import run_bass_kernel_spmd

WIN = 100
B = 512
CH = 2
T = 32768
N = T - WIN + 1
NCORES = 8
NLOC = 4084
FIN = NLOC + WIN - 1  # 4183
TPAD = (NCORES - 1) * NLOC + FIN
P = 128
NBT = B // P  # 4
NCHUNK = 4
F = NLOC // NCHUNK  # 1021
H = F + WIN  # 1121

f32 = mybir.dt.float32
bf16 = mybir.dt.bfloat16
AOT = mybir.ActivationFunctionType
ALU = mybir.AluOpType
AXL = mybir.AxisListType

NIT = NCHUNK * NBT  # 16 iterations, chunk-major: i = c*NBT + bt


def _kernel_body(tc, out, xs):
    nc = tc.nc
    import contextlib

    ctx = contextlib.ExitStack()
    with ctx:
        const_pool = ctx.enter_context(tc.tile_pool(name="const", bufs=1))
        pool = ctx.enter_context(tc.tile_pool(name="work", bufs=3))
        xpool = ctx.enter_context(tc.tile_pool(name="xin", bufs=4))
        scan_pool = ctx.enter_context(tc.tile_pool(name="scans", bufs=5))
        corr_pool = ctx.enter_context(tc.tile_pool(name="corrp", bufs=6))
        row_pool = ctx.enter_context(tc.tile_pool(name="rows", bufs=2))
        psum_pool = ctx.enter_context(tc.tile_pool(name="psum", bufs=2, space="PSUM"))

        ones = const_pool.tile([P, P], bf16, tag="ones")
        nc.vector.memset(ones[:], 1.0)
        plnw = const_pool.tile([P, 1], f32, tag="plnw")
        nc.vector.memset(plnw[:], float(np.log(WIN)))

        SQW = float(np.sqrt(WIN))

        nc.scalar.add_instruction(
            mybir.InstLoadActFuncSet(
                name=nc.get_next_instruction_name(), act_func_set_id=6
            )
        )

        # pipeline state, indexed by iteration
        S = [dict() for _ in range(NIT)]
        prev_scans = [None] * NBT  # per-bt (slin, squad, s12t) for chunk chaining
        corrs = [None] * NBT
        ps_tiles = {}

        def stage_load(i):
            c, bt = divmod(i, NBT)
            b0 = bt * P
            x12 = xpool.tile([P, CH, H], f32, tag="x12")
            g0 = 0 if c == 0 else c * F - 1
            nc.sync.dma_start(out=x12[:], in_=xs[b0 : b0 + P, :, g0 : g0 + H])
            S[i]["x12"] = x12

        def stage_pre(i):
            # sq (ACT) + p12 (Pool) from x12
            x12 = S[i]["x12"]
            sq = pool.tile([P, CH, H], bf16, tag="sq")
            nc.scalar.activation(sq[:], x12[:], AOT.Square, scale=SQW)
            p12 = pool.tile([P, H], bf16, tag="p12")
            nc.gpsimd.tensor_mul(p12[:], x12[:, 0, :], x12[:, 1, :])
            S[i]["sq"] = sq
            S[i]["p12"] = p12

        def stage_scans(i):
            c, bt = divmod(i, NBT)
            x12, sq, p12 = S[i]["x12"], S[i]["sq"], S[i]["p12"]
            slin = scan_pool.tile([P, CH, F], bf16, tag="slin")
            squad = scan_pool.tile([P, CH, F], bf16, tag="squad")
            s12t = scan_pool.tile([P, F], bf16, tag="s12t")

            def wsum(dst2d, src2d, pv):
                if c == 0:
                    with nc.allow_low_precision(reason="f32 accum, bf16 store"):
                        nc.vector.tensor_reduce(
                            out=dst2d[:, 0:1], in_=src2d[:, 0:WIN],
                            op=ALU.add, axis=AXL.X,
                        )
                    nc.vector.tensor_tensor_scan(
                        out=dst2d[:, 1:F],
                        data0=src2d[:, WIN : WIN + F - 1],
                        data1=src2d[:, 0 : F - 1],
                        initial=dst2d[:, 0:1],
                        op0=ALU.add, op1=ALU.subtract,
                    )
                else:
                    nc.vector.tensor_tensor_scan(
                        out=dst2d[:, 0:F],
                        data0=src2d[:, WIN : WIN + F],
                        data1=src2d[:, 0:F],
                        initial=pv,
                        op0=ALU.add, op1=ALU.subtract,
                    )

            pv = prev_scans[bt]
            wsum(slin[:, 0, :], x12[:, 0, :], pv and pv[0][:, 0, F - 1 : F])
            wsum(slin[:, 1, :], x12[:, 1, :], pv and pv[0][:, 1, F - 1 : F])
            wsum(squad[:, 0, :], sq[:, 0, :], pv and pv[1][:, 0, F - 1 : F])
            wsum(squad[:, 1, :], sq[:, 1, :], pv and pv[1][:, 1, F - 1 : F])
            wsum(s12t[:], p12[:], pv and pv[2][:, F - 1 : F])
            prev_scans[bt] = (slin, squad, s12t)
            S[i]["slin"], S[i]["squad"], S[i]["s12t"] = slin, squad, s12t

        def stage_mid(i):
            slin, squad, s12t = S[i]["slin"], S[i]["squad"], S[i]["s12t"]
            t = pool.tile([P, CH, F], bf16, tag="t")
            nc.scalar.activation(t[:], slin[:], AOT.Square)
            s1w = pool.tile([P, F], bf16, tag="s1w")
            nc.scalar.mul(s1w[:], slin[:, 0, :], 1.0 / WIN)
            v = pool.tile([P, CH, F], bf16, tag="v")
            nc.vector.tensor_sub(v[:], squad[:], t[:])
            m12 = pool.tile([P, F], bf16, tag="m12")
            nc.gpsimd.tensor_mul(m12[:], s1w[:], slin[:, 1, :])
            cov = pool.tile([P, F], bf16, tag="cov")
            nc.gpsimd.tensor_sub(cov[:], s12t[:], m12[:])
            p = pool.tile([P, F], bf16, tag="p")
            nc.vector.tensor_mul(p[:], v[:, 0, :], v[:, 1, :])
            S[i]["cov"], S[i]["p"] = cov, p

        def stage_tail(i):
            c, bt = divmod(i, NBT)
            cov, p = S[i]["cov"], S[i]["p"]
            lnp = pool.tile([P, F], f32, tag="lnp")
            nc.scalar.activation(lnp[:], p[:], AOT.Ln)
            rs = pool.tile([P, F], bf16, tag="rs")
            nc.scalar.activation(rs[:], lnp[:], AOT.Exp, scale=-0.5, bias=plnw[:])
            corr = corr_pool.tile([P, F], bf16, tag="corr")
            nc.vector.tensor_mul(corr[:], cov[:], rs[:])
            corrs[bt] = corr
            if bt == 0:
                ps_tiles[c] = psum_pool.tile(
                    [P, 1024], f32, tag="ps", name=f"ps{c}"
                )
            ps = ps_tiles[c]
            nc.tensor.matmul(
                ps[:, 0:512], ones[:], corr[:, 0:512],
                start=(bt == 0), stop=(bt == NBT - 1),
            )
            nc.tensor.matmul(
                ps[:, 512:F], ones[:], corr[:, 512:F],
                start=(bt == 0), stop=(bt == NBT - 1),
            )

        def stage_finalize(c):
            last = c == NCHUNK - 1
            c0 = c * F
            ps = ps_tiles.pop(c)
            avgb = row_pool.tile([P, F], bf16, tag="avgb")
            nc.scalar.mul(avgb[:], ps[:, 0:F], 1.0 / B)
            for bt in range(NBT):
                b0 = bt * P
                corr = corrs[bt]
                r = pool.tile([P, F], bf16, tag="r")
                nc.vector.tensor_sub(r[:], corr[:], avgb[:])
                if last:
                    # final chunk: ACT is on the exit critical path; DVE is idle
                    nc.vector.tensor_scalar_max(r[:], r[:], 0.0)
                else:
                    nc.scalar.activation(r[:], r[:], AOT.Relu)
                nc.sync.dma_start(out=out[b0 : b0 + P, c0 : c0 + F], in_=r[:])

        # software-pipelined emission; tail is delayed one iteration so the
        # ACT queue orders t/s1w(i) ahead of ln/rs(i-1) (avoids head-of-line
        # blocking on the Pool p(i-1) dependency)
        stage_load(0)
        stage_pre(0)
        stage_load(1)
        for i in range(NIT):
            if i + 1 < NIT:
                stage_pre(i + 1)
            if i + 2 < NIT:
                stage_load(i + 2)
            stage_scans(i)
            stage_mid(i)
            if i >= 1:
                stage_tail(i - 1)
                c, bt = divmod(i - 1, NBT)
                if bt == NBT - 1:
                    stage_finalize(c)
        stage_tail(NIT - 1)
        stage_finalize(NCHUNK - 1)


def build_nc():
    from concourse import bacc

    nc = bacc.Bacc("TRN2", target_bir_lowering=False, debug=False, num_devices=NCORES)
    xs = nc.dram_tensor("xs", [B, CH, FIN], f32, kind="ExternalInput").ap()
    out = nc.dram_tensor("out", [B, NLOC], bf16, kind="ExternalOutput").ap()
    with tile.TileContext(nc) as tc:
        _kernel_body(tc, out, xs)
    nc.compile()
    return nc


_NC = None


def _get_nc():
    global _NC
    if _NC is None:
        _NC = build_nc()
    return _NC


def make_in_maps(x):
    x = np.asarray(x, dtype=np.float32)
    xpad = np.zeros((B, CH, TPAD), dtype=np.float32)
    xpad[:, :, :T] = x
    return [
        {"xs": np.ascontiguousarray(xpad[:, :, c * NLOC : c * NLOC + FIN])}
        for c in range(NCORES)
    ]


def _run(x, **kwargs):
    nc = _get_nc()
    res = run_bass_kernel_spmd(nc, make_in_maps(x), core_ids=list(range(NCORES)), **kwargs)
    outs = [np.asarray(res.results[c]["out"]).astype(np.float32) for c in range(NCORES)]
    full = np.concatenate(outs, axis=1)[:, :N]
    return full, res


def kernel(x):
    full, _ = _run(x)
    return full


# revision 5
# speedup vs baseline: 1.7308x; 1.0362x over previous
"""Trainium2 Bass kernel (final): sliding-window Pearson correlation attention.

Same math/precision scheme as v3 (bf16 pipeline, f32 ln link, DVE-only scans,
act-table preload), plus software-pipelined emission: each iteration emits the
NEXT iteration's DMA/sq/p12 before the current iteration's dependent tail, so
the in-order engine queues never sit behind a cross-engine wait with runnable
work elsewhere in the program order.

Engine split per (chunk, batch-tile) iteration (model ~8.5us):
  DVE:  5 scans, v, m12, cov, corr
  ACT:  sq, t, s1w, ln, rs (+avgb per chunk)
  Pool: p12, p, r, relu
  PE:   batch-sum matmuls (bf16, M=128 replicated)
"""

import numpy as np

import concourse.bass as bass
import concourse.mybir as mybir
import concourse.tile as tile
from concourse.bass_utils import run_bass_kernel_spmd

WIN = 100
B = 512
CH = 2
T = 32768
N = T - WIN + 1
NCORES = 8
NLOC = 4084
FIN = NLOC + WIN - 1  # 4183
TPAD = (NCORES - 1) * NLOC + FIN
P = 128
NBT = B // P  # 4
NCHUNK = 4
F = NLOC // NCHUNK  # 1021
H = F + WIN  # 1121

f32 = mybir.dt.float32
bf16 = mybir.dt.bfloat16
AOT = mybir.ActivationFunctionType
ALU = mybir.AluOpType
AXL = mybir.AxisListType

NIT = NCHUNK * NBT  # 16 iterations, chunk-major: i = c*NBT + bt


def _kernel_body(tc, out, xs):
    nc = tc.nc
    import contextlib

    ctx = contextlib.ExitStack()
    with ctx:
        const_pool = ctx.enter_context(tc.tile_pool(name="const", bufs=1))
        pool = ctx.enter_context(tc.tile_pool(name="work", bufs=3))
        xpool = ctx.enter_context(tc.tile_pool(name="xin", bufs=4))
        scan_pool = ctx.enter_context(tc.tile_pool(name="scans", bufs=5))
        corr_pool = ctx.enter_context(tc.tile_pool(name="corrp", bufs=6))
        row_pool = ctx.enter_context(tc.tile_pool(name="rows", bufs=2))
        psum_pool = ctx.enter_context(tc.tile_pool(name="psum", bufs=2, space="PSUM"))
        psr_pool = ctx.enter_context(tc.tile_pool(name="psumr", bufs=2, space="PSUM"))

        ones = const_pool.tile([P, P], bf16, tag="ones")
        nc.vector.memset(ones[:], 1.0)
        # identity matrix: keep ones where (free_idx - partition_idx) == 0
        ident = const_pool.tile([P, P], bf16, tag="ident")
        nc.gpsimd.affine_select(
            ident[:], ones[:], pattern=[[1, P]],
            compare_op=ALU.is_equal, fill=0.0, base=0, channel_multiplier=-1,
        )
        plnw = const_pool.tile([P, 1], f32, tag="plnw")
        nc.vector.memset(plnw[:], float(np.log(WIN)))

        SQW = float(np.sqrt(WIN))

        nc.scalar.add_instruction(
            mybir.InstLoadActFuncSet(
                name=nc.get_next_instruction_name(), act_func_set_id=6
            )
        )

        # pipeline state, indexed by iteration
        S = [dict() for _ in range(NIT)]
        prev_scans = [None] * NBT  # per-bt (slin, squad, s12t) for chunk chaining
        corrs = [None] * NBT
        ps_tiles = {}

        def stage_load(i):
            c, bt = divmod(i, NBT)
            b0 = bt * P
            x12 = xpool.tile([P, CH, H], f32, tag="x12")
            g0 = 0 if c == 0 else c * F - 1
            nc.sync.dma_start(out=x12[:], in_=xs[b0 : b0 + P, :, g0 : g0 + H])
            S[i]["x12"] = x12

        def stage_pre(i):
            # sq (ACT) + p12 (Pool) from x12
            x12 = S[i]["x12"]
            sq = pool.tile([P, CH, H], bf16, tag="sq")
            nc.scalar.activation(sq[:], x12[:], AOT.Square, scale=SQW)
            p12 = pool.tile([P, H], bf16, tag="p12")
            nc.gpsimd.tensor_mul(p12[:], x12[:, 0, :], x12[:, 1, :])
            S[i]["sq"] = sq
            S[i]["p12"] = p12

        def stage_scans(i):
            c, bt = divmod(i, NBT)
            x12, sq, p12 = S[i]["x12"], S[i]["sq"], S[i]["p12"]
            slin = scan_pool.tile([P, CH, F], bf16, tag="slin")
            squad = scan_pool.tile([P, CH, F], bf16, tag="squad")
            s12t = scan_pool.tile([P, F], bf16, tag="s12t")

            def wsum(dst2d, src2d, pv):
                if c == 0:
                    with nc.allow_low_precision(reason="f32 accum, bf16 store"):
                        nc.vector.tensor_reduce(
                            out=dst2d[:, 0:1], in_=src2d[:, 0:WIN],
                            op=ALU.add, axis=AXL.X,
                        )
                    nc.vector.tensor_tensor_scan(
                        out=dst2d[:, 1:F],
                        data0=src2d[:, WIN : WIN + F - 1],
                        data1=src2d[:, 0 : F - 1],
                        initial=dst2d[:, 0:1],
                        op0=ALU.add, op1=ALU.subtract,
                    )
                else:
                    nc.vector.tensor_tensor_scan(
                        out=dst2d[:, 0:F],
                        data0=src2d[:, WIN : WIN + F],
                        data1=src2d[:, 0:F],
                        initial=pv,
                        op0=ALU.add, op1=ALU.subtract,
                    )

            pv = prev_scans[bt]
            wsum(slin[:, 0, :], x12[:, 0, :], pv and pv[0][:, 0, F - 1 : F])
            wsum(slin[:, 1, :], x12[:, 1, :], pv and pv[0][:, 1, F - 1 : F])
            wsum(squad[:, 0, :], sq[:, 0, :], pv and pv[1][:, 0, F - 1 : F])
            wsum(squad[:, 1, :], sq[:, 1, :], pv and pv[1][:, 1, F - 1 : F])
            wsum(s12t[:], p12[:], pv and pv[2][:, F - 1 : F])
            prev_scans[bt] = (slin, squad, s12t)
            S[i]["slin"], S[i]["squad"], S[i]["s12t"] = slin, squad, s12t

        def stage_mid(i):
            slin, squad, s12t = S[i]["slin"], S[i]["squad"], S[i]["s12t"]
            t = pool.tile([P, CH, F], bf16, tag="t")
            nc.scalar.activation(t[:], slin[:], AOT.Square)
            s1w = pool.tile([P, F], bf16, tag="s1w")
            nc.gpsimd.tensor_scalar_mul(s1w[:], slin[:, 0, :], 1.0 / WIN)
            v = pool.tile([P, CH, F], bf16, tag="v")
            nc.vector.tensor_sub(v[:], squad[:], t[:])
            m12 = pool.tile([P, F], bf16, tag="m12")
            nc.gpsimd.tensor_mul(m12[:], s1w[:], slin[:, 1, :])
            cov = pool.tile([P, F], bf16, tag="cov")
            nc.gpsimd.tensor_sub(cov[:], s12t[:], m12[:])
            p = pool.tile([P, F], bf16, tag="p")
            nc.vector.tensor_mul(p[:], v[:, 0, :], v[:, 1, :])
            S[i]["cov"], S[i]["p"] = cov, p

        def stage_tail(i):
            c, bt = divmod(i, NBT)
            cov, p = S[i]["cov"], S[i]["p"]
            lnp = pool.tile([P, F], f32, tag="lnp")
            nc.scalar.activation(lnp[:], p[:], AOT.Ln)
            rs = pool.tile([P, F], bf16, tag="rs")
            nc.scalar.activation(rs[:], lnp[:], AOT.Exp, scale=-0.5, bias=plnw[:])
            corr = corr_pool.tile([P, F], bf16, tag="corr")
            nc.vector.tensor_mul(corr[:], cov[:], rs[:])
            corrs[bt] = corr
            if bt == 0:
                ps_tiles[c] = psum_pool.tile(
                    [P, 1024], f32, tag="ps", name=f"ps{c}"
                )
            ps = ps_tiles[c]
            nc.tensor.matmul(
                ps[:, 0:512], ones[:], corr[:, 0:512],
                start=(bt == 0), stop=(bt == NBT - 1),
            )
            nc.tensor.matmul(
                ps[:, 512:F], ones[:], corr[:, 512:F],
                start=(bt == 0), stop=(bt == NBT - 1),
            )

        def stage_finalize(c):
            last = c == NCHUNK - 1
            c0 = c * F
            ps = ps_tiles.pop(c)
            avgb = row_pool.tile([P, F], bf16, tag="avgb")
            nc.scalar.mul(avgb[:], ps[:, 0:F], -1.0 / B)
            for bt in range(NBT):
                b0 = bt * P
                corr = corrs[bt]
                # r = corr - mean on the PE: identity-matmul accumulate
                psr = psr_pool.tile([P, 1024], f32, tag="psr", name=f"psr{c}_{bt}")
                for lo, hi in ((0, 512), (512, F)):
                    nc.tensor.matmul(
                        psr[:, lo:hi], ident[:], corr[:, lo:hi],
                        start=True, stop=False,
                    )
                    nc.tensor.matmul(
                        psr[:, lo:hi], ident[:], avgb[:, lo:hi],
                        start=False, stop=True,
                    )
                r = pool.tile([P, F], bf16, tag="r")
                if last:
                    nc.vector.tensor_scalar_max(r[:], psr[:, 0:F], 0.0)
                else:
                    nc.scalar.activation(r[:], psr[:, 0:F], AOT.Relu)
                nc.sync.dma_start(out=out[b0 : b0 + P, c0 : c0 + F], in_=r[:])

        # software-pipelined emission; tail is delayed one iteration so the
        # ACT queue orders t/s1w(i) ahead of ln/rs(i-1) (avoids head-of-line
        # blocking on the Pool p(i-1) dependency)
        stage_load(0)
        stage_pre(0)
        stage_load(1)
        for i in range(NIT):
            if i + 1 < NIT:
                stage_pre(i + 1)
            if i + 2 < NIT:
                stage_load(i + 2)
            stage_scans(i)
            stage_mid(i)
            if i >= 1:
                stage_tail(i - 1)
                c, bt = divmod(i - 1, NBT)
                if bt == NBT - 1:
                    stage_finalize(c)
        stage_tail(NIT - 1)
        stage_finalize(NCHUNK - 1)


def build_nc():
    from concourse import bacc

    nc = bacc.Bacc("TRN2", target_bir_lowering=False, debug=False, num_devices=NCORES)
    xs = nc.dram_tensor("xs", [B, CH, FIN], f32, kind="ExternalInput").ap()
    out = nc.dram_tensor("out", [B, NLOC], bf16, kind="ExternalOutput").ap()
    with tile.TileContext(nc) as tc:
        _kernel_body(tc, out, xs)
    nc.compile()
    return nc


_NC = None


def _get_nc():
    global _NC
    if _NC is None:
        _NC = build_nc()
    return _NC


def make_in_maps(x):
    x = np.asarray(x, dtype=np.float32)
    xpad = np.zeros((B, CH, TPAD), dtype=np.float32)
    xpad[:, :, :T] = x
    return [
        {"xs": np.ascontiguousarray(xpad[:, :, c * NLOC : c * NLOC + FIN])}
        for c in range(NCORES)
    ]


def _run(x, **kwargs):
    nc = _get_nc()
    res = run_bass_kernel_spmd(nc, make_in_maps(x), core_ids=list(range(NCORES)), **kwargs)
    outs = [np.asarray(res.results[c]["out"]).astype(np.float32) for c in range(NCORES)]
    full = np.concatenate(outs, axis=1)[:, :N]
    return full, res


def kernel(x):
    full, _ = _run(x)
    return full


# revision 6
# speedup vs baseline: 1.7328x; 1.0012x over previous
"""Trainium2 Bass kernel (final): sliding-window Pearson correlation attention.

Same math/precision scheme as v3 (bf16 pipeline, f32 ln link, DVE-only scans,
act-table preload), plus software-pipelined emission: each iteration emits the
NEXT iteration's DMA/sq/p12 before the current iteration's dependent tail, so
the in-order engine queues never sit behind a cross-engine wait with runnable
work elsewhere in the program order.

Engine split per (chunk, batch-tile) iteration (model ~8.5us):
  DVE:  5 scans, v, m12, cov, corr
  ACT:  sq, t, s1w, ln, rs (+avgb per chunk)
  Pool: p12, p, r, relu
  PE:   batch-sum matmuls (bf16, M=128 replicated)
"""

import numpy as np

import concourse.bass as bass
import concourse.mybir as mybir
import concourse.tile as tile
from concourse.bass_utils import run_bass_kernel_spmd

WIN = 100
B = 512
CH = 2
T = 32768
N = T - WIN + 1
NCORES = 8
NLOC = 4084
FIN = NLOC + WIN - 1  # 4183
TPAD = (NCORES - 1) * NLOC + FIN
P = 128
NBT = B // P  # 4
NCHUNK = 4
F = NLOC // NCHUNK  # 1021
H = F + WIN  # 1121

f32 = mybir.dt.float32
bf16 = mybir.dt.bfloat16
AOT = mybir.ActivationFunctionType
ALU = mybir.AluOpType
AXL = mybir.AxisListType

NIT = NCHUNK * NBT  # 16 iterations, chunk-major: i = c*NBT + bt


def _kernel_body(tc, out, xs):
    nc = tc.nc
    import contextlib

    ctx = contextlib.ExitStack()
    with ctx:
        const_pool = ctx.enter_context(tc.tile_pool(name="const", bufs=1))
        pool = ctx.enter_context(tc.tile_pool(name="work", bufs=3))
        xpool = ctx.enter_context(tc.tile_pool(name="xin", bufs=4))
        scan_pool = ctx.enter_context(tc.tile_pool(name="scans", bufs=5))
        corr_pool = ctx.enter_context(tc.tile_pool(name="corrp", bufs=6))
        row_pool = ctx.enter_context(tc.tile_pool(name="rows", bufs=2))
        psum_pool = ctx.enter_context(tc.tile_pool(name="psum", bufs=2, space="PSUM"))
        psr_pool = ctx.enter_context(tc.tile_pool(name="psumr", bufs=2, space="PSUM"))

        ones = const_pool.tile([P, P], bf16, tag="ones")
        nc.vector.memset(ones[:], 1.0)
        # identity matrix: keep ones where (free_idx - partition_idx) == 0
        ident = const_pool.tile([P, P], bf16, tag="ident")
        nc.gpsimd.affine_select(
            ident[:], ones[:], pattern=[[1, P]],
            compare_op=ALU.is_equal, fill=0.0, base=0, channel_multiplier=-1,
        )
        plnw = const_pool.tile([P, 1], f32, tag="plnw")
        nc.vector.memset(plnw[:], float(np.log(WIN)))

        SQW = float(np.sqrt(WIN))

        nc.scalar.add_instruction(
            mybir.InstLoadActFuncSet(
                name=nc.get_next_instruction_name(), act_func_set_id=6
            )
        )

        # pipeline state, indexed by iteration
        S = [dict() for _ in range(NIT)]
        prev_scans = [None] * NBT  # per-bt (slin, squad, s12t) for chunk chaining
        corrs = [None] * NBT
        ps_tiles = {}

        def stage_load(i):
            c, bt = divmod(i, NBT)
            b0 = bt * P
            x12 = xpool.tile([P, CH, H], f32, tag="x12")
            g0 = 0 if c == 0 else c * F - 1
            nc.sync.dma_start(out=x12[:], in_=xs[b0 : b0 + P, :, g0 : g0 + H])
            S[i]["x12"] = x12

        def stage_pre(i):
            # sq (ACT) + p12 (Pool) from x12
            x12 = S[i]["x12"]
            sq = pool.tile([P, CH, H], bf16, tag="sq")
            nc.scalar.activation(sq[:], x12[:], AOT.Square, scale=SQW)
            p12 = pool.tile([P, H], bf16, tag="p12")
            nc.gpsimd.tensor_mul(p12[:], x12[:, 0, :], x12[:, 1, :])
            S[i]["sq"] = sq
            S[i]["p12"] = p12

        def stage_scans(i):
            c, bt = divmod(i, NBT)
            x12, sq, p12 = S[i]["x12"], S[i]["sq"], S[i]["p12"]
            slin = scan_pool.tile([P, CH, F], bf16, tag="slin")
            squad = scan_pool.tile([P, CH, F], bf16, tag="squad")
            s12t = scan_pool.tile([P, F], bf16, tag="s12t")

            def wsum(dst2d, src2d, pv):
                if c == 0:
                    with nc.allow_low_precision(reason="f32 accum, bf16 store"):
                        nc.vector.tensor_reduce(
                            out=dst2d[:, 0:1], in_=src2d[:, 0:WIN],
                            op=ALU.add, axis=AXL.X,
                        )
                    nc.vector.tensor_tensor_scan(
                        out=dst2d[:, 1:F],
                        data0=src2d[:, WIN : WIN + F - 1],
                        data1=src2d[:, 0 : F - 1],
                        initial=dst2d[:, 0:1],
                        op0=ALU.add, op1=ALU.subtract,
                    )
                else:
                    nc.vector.tensor_tensor_scan(
                        out=dst2d[:, 0:F],
                        data0=src2d[:, WIN : WIN + F],
                        data1=src2d[:, 0:F],
                        initial=pv,
                        op0=ALU.add, op1=ALU.subtract,
                    )

            pv = prev_scans[bt]
            wsum(slin[:, 0, :], x12[:, 0, :], pv and pv[0][:, 0, F - 1 : F])
            wsum(slin[:, 1, :], x12[:, 1, :], pv and pv[0][:, 1, F - 1 : F])
            wsum(squad[:, 0, :], sq[:, 0, :], pv and pv[1][:, 0, F - 1 : F])
            wsum(squad[:, 1, :], sq[:, 1, :], pv and pv[1][:, 1, F - 1 : F])
            wsum(s12t[:], p12[:], pv and pv[2][:, F - 1 : F])
            prev_scans[bt] = (slin, squad, s12t)
            S[i]["slin"], S[i]["squad"], S[i]["s12t"] = slin, squad, s12t

        def stage_mid(i):
            slin, squad, s12t = S[i]["slin"], S[i]["squad"], S[i]["s12t"]
            t = pool.tile([P, CH, F], bf16, tag="t")
            nc.scalar.activation(t[:], slin[:], AOT.Square)
            s1w = pool.tile([P, F], bf16, tag="s1w")
            nc.gpsimd.tensor_scalar_mul(s1w[:], slin[:, 0, :], 1.0 / WIN)
            v = pool.tile([P, CH, F], bf16, tag="v")
            nc.vector.tensor_sub(v[:], squad[:], t[:])
            m12 = pool.tile([P, F], bf16, tag="m12")
            nc.gpsimd.tensor_mul(m12[:], s1w[:], slin[:, 1, :])
            cov = pool.tile([P, F], bf16, tag="cov")
            nc.gpsimd.tensor_sub(cov[:], s12t[:], m12[:])
            p = pool.tile([P, F], bf16, tag="p")
            nc.vector.tensor_mul(p[:], v[:, 0, :], v[:, 1, :])
            S[i]["cov"], S[i]["p"] = cov, p

        def stage_tail(i):
            c, bt = divmod(i, NBT)
            cov, p = S[i]["cov"], S[i]["p"]
            lnp = pool.tile([P, F], f32, tag="lnp")
            nc.scalar.activation(lnp[:], p[:], AOT.Ln)
            rs = pool.tile([P, F], bf16, tag="rs")
            nc.scalar.activation(rs[:], lnp[:], AOT.Exp, scale=-0.5, bias=plnw[:])
            corr = corr_pool.tile([P, F], bf16, tag="corr")
            nc.vector.tensor_mul(corr[:], cov[:], rs[:])
            corrs[bt] = corr
            if bt == 0:
                ps_tiles[c] = psum_pool.tile(
                    [P, 1024], f32, tag="ps", name=f"ps{c}"
                )
            ps = ps_tiles[c]
            nc.tensor.matmul(
                ps[:, 0:512], ones[:], corr[:, 0:512],
                start=(bt == 0), stop=(bt == NBT - 1),
            )
            nc.tensor.matmul(
                ps[:, 512:F], ones[:], corr[:, 512:F],
                start=(bt == 0), stop=(bt == NBT - 1),
            )

        def stage_finalize(c):
            last = c == NCHUNK - 1
            c0 = c * F
            ps = ps_tiles.pop(c)
            avgb = row_pool.tile([P, F], bf16, tag="avgb")
            nc.scalar.mul(avgb[:, 0:512], ps[:, 0:512], -1.0 / B)
            nc.scalar.mul(avgb[:, 512:F], ps[:, 512:F], -1.0 / B)
            for bt in range(NBT):
                b0 = bt * P
                corr = corrs[bt]
                # r = corr - mean on the PE: identity-matmul accumulate
                psr = psr_pool.tile([P, 1024], f32, tag="psr", name=f"psr{c}_{bt}")
                for lo, hi in ((0, 512), (512, F)):
                    nc.tensor.matmul(
                        psr[:, lo:hi], ident[:], corr[:, lo:hi],
                        start=True, stop=False,
                    )
                    nc.tensor.matmul(
                        psr[:, lo:hi], ident[:], avgb[:, lo:hi],
                        start=False, stop=True,
                    )
                r = pool.tile([P, F], bf16, tag="r")
                if last and bt % 2 == 1:
                    nc.vector.tensor_scalar_max(r[:], psr[:, 0:F], 0.0)
                else:
                    nc.scalar.activation(r[:], psr[:, 0:F], AOT.Relu)
                nc.sync.dma_start(out=out[b0 : b0 + P, c0 : c0 + F], in_=r[:])

        # software-pipelined emission; tail is delayed one iteration so the
        # ACT queue orders t/s1w(i) ahead of ln/rs(i-1) (avoids head-of-line
        # blocking on the Pool p(i-1) dependency)
        stage_load(0)
        stage_pre(0)
        stage_load(1)
        for i in range(NIT):
            if i + 1 < NIT:
                stage_pre(i + 1)
            if i + 2 < NIT:
                stage_load(i + 2)
            stage_scans(i)
            stage_mid(i)
            if i >= 1:
                stage_tail(i - 1)
                c, bt = divmod(i - 1, NBT)
                if bt == NBT - 1:
                    stage_finalize(c)
        stage_tail(NIT - 1)
        stage_finalize(NCHUNK - 1)


def build_nc():
    from concourse import bacc

    nc = bacc.Bacc("TRN2", target_bir_lowering=False, debug=False, num_devices=NCORES)
    xs = nc.dram_tensor("xs", [B, CH, FIN], f32, kind="ExternalInput").ap()
    out = nc.dram_tensor("out", [B, NLOC], bf16, kind="ExternalOutput").ap()
    with tile.TileContext(nc) as tc:
        _kernel_body(tc, out, xs)
    nc.compile()
    return nc


_NC = None


def _get_nc():
    global _NC
    if _NC is None:
        _NC = build_nc()
    return _NC


def make_in_maps(x):
    x = np.asarray(x, dtype=np.float32)
    xpad = np.zeros((B, CH, TPAD), dtype=np.float32)
    xpad[:, :, :T] = x
    return [
        {"xs": np.ascontiguousarray(xpad[:, :, c * NLOC : c * NLOC + FIN])}
        for c in range(NCORES)
    ]


def _run(x, **kwargs):
    nc = _get_nc()
    res = run_bass_kernel_spmd(nc, make_in_maps(x), core_ids=list(range(NCORES)), **kwargs)
    outs = [np.asarray(res.results[c]["out"]).astype(np.float32) for c in range(NCORES)]
    full = np.concatenate(outs, axis=1)[:, :N]
    return full, res


def kernel(x):
    full, _ = _run(x)
    return full


# revision 7
# speedup vs baseline: 1.7507x; 1.0104x over previous
"""Trainium2 Bass kernel (final): sliding-window Pearson correlation attention.

Same math/precision scheme as v3 (bf16 pipeline, f32 ln link, DVE-only scans,
act-table preload), plus software-pipelined emission: each iteration emits the
NEXT iteration's DMA/sq/p12 before the current iteration's dependent tail, so
the in-order engine queues never sit behind a cross-engine wait with runnable
work elsewhere in the program order.

Engine split per (chunk, batch-tile) iteration (model ~8.5us):
  DVE:  5 scans, v, m12, cov, corr
  ACT:  sq, t, s1w, ln, rs (+avgb per chunk)
  Pool: p12, p, r, relu
  PE:   batch-sum matmuls (bf16, M=128 replicated)
"""

import numpy as np

import concourse.bass as bass
import concourse.mybir as mybir
import concourse.tile as tile
from concourse.bass_utils import run_bass_kernel_spmd

WIN = 100
B = 512
CH = 2
T = 32768
N = T - WIN + 1
NCORES = 8
NLOC = 4084
FIN = NLOC + WIN - 1  # 4183
TPAD = (NCORES - 1) * NLOC + FIN
P = 128
NBT = B // P  # 4
NCHUNK = 4
F = NLOC // NCHUNK  # 1021
H = F + WIN  # 1121

f32 = mybir.dt.float32
bf16 = mybir.dt.bfloat16
AOT = mybir.ActivationFunctionType
ALU = mybir.AluOpType
AXL = mybir.AxisListType

NIT = NCHUNK * NBT  # 16 iterations, chunk-major: i = c*NBT + bt


def _kernel_body(tc, out, xs):
    nc = tc.nc
    import contextlib

    ctx = contextlib.ExitStack()
    with ctx:
        const_pool = ctx.enter_context(tc.tile_pool(name="const", bufs=1))
        pool = ctx.enter_context(tc.tile_pool(name="work", bufs=3))
        xpool = ctx.enter_context(tc.tile_pool(name="xin", bufs=4))
        scan_pool = ctx.enter_context(tc.tile_pool(name="scans", bufs=5))
        corr_pool = ctx.enter_context(tc.tile_pool(name="corrp", bufs=6))
        row_pool = ctx.enter_context(tc.tile_pool(name="rows", bufs=3))
        psum_pool = ctx.enter_context(tc.tile_pool(name="psum", bufs=2, space="PSUM"))
        psr_pool = ctx.enter_context(tc.tile_pool(name="psumr", bufs=2, space="PSUM"))

        ones = const_pool.tile([P, P], bf16, tag="ones")
        nc.vector.memset(ones[:], 1.0)
        # identity matrix: keep ones where (free_idx - partition_idx) == 0
        ident = const_pool.tile([P, P], bf16, tag="ident")
        nc.gpsimd.affine_select(
            ident[:], ones[:], pattern=[[1, P]],
            compare_op=ALU.is_equal, fill=0.0, base=0, channel_multiplier=-1,
        )
        plnw = const_pool.tile([P, 1], f32, tag="plnw")
        nc.vector.memset(plnw[:], float(np.log(WIN)))

        SQW = float(np.sqrt(WIN))

        nc.scalar.add_instruction(
            mybir.InstLoadActFuncSet(
                name=nc.get_next_instruction_name(), act_func_set_id=6
            )
        )

        # pipeline state, indexed by iteration
        S = [dict() for _ in range(NIT)]
        prev_scans = [None] * NBT  # per-bt (slin, squad, s12t) for chunk chaining
        corrs = [None] * NBT
        ps_tiles = {}
        avgb_tiles = {}
        corr_final = {}

        def stage_load(i):
            c, bt = divmod(i, NBT)
            b0 = bt * P
            x12 = xpool.tile([P, CH, H], f32, tag="x12")
            g0 = 0 if c == 0 else c * F - 1
            nc.sync.dma_start(out=x12[:], in_=xs[b0 : b0 + P, :, g0 : g0 + H])
            S[i]["x12"] = x12

        def stage_pre(i):
            # sq (ACT) + p12 (Pool) from x12
            x12 = S[i]["x12"]
            sq = pool.tile([P, CH, H], bf16, tag="sq")
            nc.scalar.activation(sq[:], x12[:], AOT.Square, scale=SQW)
            p12 = pool.tile([P, H], bf16, tag="p12")
            nc.gpsimd.tensor_mul(p12[:], x12[:, 0, :], x12[:, 1, :])
            S[i]["sq"] = sq
            S[i]["p12"] = p12

        def stage_scans(i):
            c, bt = divmod(i, NBT)
            x12, sq, p12 = S[i]["x12"], S[i]["sq"], S[i]["p12"]
            slin = scan_pool.tile([P, CH, F], bf16, tag="slin")
            squad = scan_pool.tile([P, CH, F], bf16, tag="squad")
            s12t = scan_pool.tile([P, F], bf16, tag="s12t")

            def wsum(dst2d, src2d, pv):
                if c == 0:
                    with nc.allow_low_precision(reason="f32 accum, bf16 store"):
                        nc.vector.tensor_reduce(
                            out=dst2d[:, 0:1], in_=src2d[:, 0:WIN],
                            op=ALU.add, axis=AXL.X,
                        )
                    nc.vector.tensor_tensor_scan(
                        out=dst2d[:, 1:F],
                        data0=src2d[:, WIN : WIN + F - 1],
                        data1=src2d[:, 0 : F - 1],
                        initial=dst2d[:, 0:1],
                        op0=ALU.add, op1=ALU.subtract,
                    )
                else:
                    nc.vector.tensor_tensor_scan(
                        out=dst2d[:, 0:F],
                        data0=src2d[:, WIN : WIN + F],
                        data1=src2d[:, 0:F],
                        initial=pv,
                        op0=ALU.add, op1=ALU.subtract,
                    )

            pv = prev_scans[bt]
            wsum(slin[:, 0, :], x12[:, 0, :], pv and pv[0][:, 0, F - 1 : F])
            wsum(slin[:, 1, :], x12[:, 1, :], pv and pv[0][:, 1, F - 1 : F])
            wsum(squad[:, 0, :], sq[:, 0, :], pv and pv[1][:, 0, F - 1 : F])
            wsum(squad[:, 1, :], sq[:, 1, :], pv and pv[1][:, 1, F - 1 : F])
            wsum(s12t[:], p12[:], pv and pv[2][:, F - 1 : F])
            prev_scans[bt] = (slin, squad, s12t)
            S[i]["slin"], S[i]["squad"], S[i]["s12t"] = slin, squad, s12t

        def stage_mid(i):
            slin, squad, s12t = S[i]["slin"], S[i]["squad"], S[i]["s12t"]
            t = pool.tile([P, CH, F], bf16, tag="t")
            nc.scalar.activation(t[:], slin[:], AOT.Square)
            s1w = pool.tile([P, F], bf16, tag="s1w")
            nc.scalar.mul(s1w[:], slin[:, 0, :], 1.0 / WIN)
            v = pool.tile([P, CH, F], bf16, tag="v")
            nc.vector.tensor_sub(v[:], squad[:], t[:])
            m12 = pool.tile([P, F], bf16, tag="m12")
            nc.gpsimd.tensor_mul(m12[:], s1w[:], slin[:, 1, :])
            cov = pool.tile([P, F], bf16, tag="cov")
            nc.gpsimd.tensor_sub(cov[:], s12t[:], m12[:])
            p = pool.tile([P, F], bf16, tag="p")
            nc.vector.tensor_mul(p[:], v[:, 0, :], v[:, 1, :])
            S[i]["cov"], S[i]["p"] = cov, p

        def stage_tail(i):
            c, bt = divmod(i, NBT)
            cov, p = S[i]["cov"], S[i]["p"]
            lnp = pool.tile([P, F], f32, tag="lnp")
            nc.scalar.activation(lnp[:], p[:], AOT.Ln)
            rs = pool.tile([P, F], bf16, tag="rs")
            nc.scalar.activation(rs[:], lnp[:], AOT.Exp, scale=-0.5, bias=plnw[:])
            corr = corr_pool.tile([P, F], bf16, tag="corr")
            nc.vector.tensor_mul(corr[:], cov[:], rs[:])
            corrs[bt] = corr
            corr_final[(c, bt)] = corr
            if bt == 0:
                ps_tiles[c] = psum_pool.tile(
                    [P, 1024], f32, tag="ps", name=f"ps{c}"
                )
            ps = ps_tiles[c]
            nc.tensor.matmul(
                ps[:, 0:512], ones[:], corr[:, 0:512],
                start=(bt == 0), stop=(bt == NBT - 1),
            )
            nc.tensor.matmul(
                ps[:, 512:F], ones[:], corr[:, 512:F],
                start=(bt == 0), stop=(bt == NBT - 1),
            )

        def stage_finalize_avgb(c):
            ps = ps_tiles.pop(c)
            avgb = row_pool.tile([P, F], bf16, tag="avgb", name=f"avgb{c}")
            nc.scalar.mul(avgb[:, 0:512], ps[:, 0:512], -1.0 / B)
            nc.scalar.mul(avgb[:, 512:F], ps[:, 512:F], -1.0 / B)
            avgb_tiles[c] = avgb

        def stage_finalize_bt(c, bt, last):
            c0 = c * F
            avgb = avgb_tiles[c]
            b0 = bt * P
            corr = corr_final[(c, bt)]
            # r = corr - mean on the PE: identity-matmul accumulate
            psr = psr_pool.tile([P, 1024], f32, tag="psr", name=f"psr{c}_{bt}")
            for lo, hi in ((0, 512), (512, F)):
                nc.tensor.matmul(
                    psr[:, lo:hi], ident[:], corr[:, lo:hi],
                    start=True, stop=False,
                )
                nc.tensor.matmul(
                    psr[:, lo:hi], ident[:], avgb[:, lo:hi],
                    start=False, stop=True,
                )
            r = pool.tile([P, F], bf16, tag="r")
            if last and bt % 2 == 1:
                nc.vector.tensor_scalar_max(r[:], psr[:, 0:F], 0.0)
            else:
                nc.scalar.activation(r[:], psr[:, 0:F], AOT.Relu)
            nc.sync.dma_start(out=out[b0 : b0 + P, c0 : c0 + F], in_=r[:])

        # software-pipelined emission; tail is delayed one iteration so the
        # ACT queue orders t/s1w(i) ahead of ln/rs(i-1) (avoids head-of-line
        # blocking on the Pool p(i-1) dependency)
        fin_q = []  # (c, bt, last) pending spread finalizes

        def pump_finalize(n):
            for _ in range(min(n, len(fin_q))):
                stage_finalize_bt(*fin_q.pop(0))

        stage_load(0)
        stage_pre(0)
        stage_load(1)
        for i in range(NIT):
            if i + 1 < NIT:
                stage_pre(i + 1)
            if i + 2 < NIT:
                stage_load(i + 2)
            stage_scans(i)
            stage_mid(i)
            pump_finalize(1)
            if i >= 1:
                stage_tail(i - 1)
                c, bt = divmod(i - 1, NBT)
                if bt == NBT - 1:
                    stage_finalize_avgb(c)
                    fin_q.extend((c, b, c == NCHUNK - 1) for b in range(NBT))
        stage_tail(NIT - 1)
        stage_finalize_avgb(NCHUNK - 1)
        fin_q.extend((NCHUNK - 1, b, True) for b in range(NBT))
        pump_finalize(len(fin_q))


def build_nc():
    from concourse import bacc

    nc = bacc.Bacc("TRN2", target_bir_lowering=False, debug=False, num_devices=NCORES)
    xs = nc.dram_tensor("xs", [B, CH, FIN], f32, kind="ExternalInput").ap()
    out = nc.dram_tensor("out", [B, NLOC], bf16, kind="ExternalOutput").ap()
    with tile.TileContext(nc) as tc:
        _kernel_body(tc, out, xs)
    nc.compile()
    return nc


_NC = None


def _get_nc():
    global _NC
    if _NC is None:
        _NC = build_nc()
    return _NC


def make_in_maps(x):
    x = np.asarray(x, dtype=np.float32)
    xpad = np.zeros((B, CH, TPAD), dtype=np.float32)
    xpad[:, :, :T] = x
    return [
        {"xs": np.ascontiguousarray(xpad[:, :, c * NLOC : c * NLOC + FIN])}
        for c in range(NCORES)
    ]


def _run(x, **kwargs):
    nc = _get_nc()
    res = run_bass_kernel_spmd(nc, make_in_maps(x), core_ids=list(range(NCORES)), **kwargs)
    outs = [np.asarray(res.results[c]["out"]).astype(np.float32) for c in range(NCORES)]
    full = np.concatenate(outs, axis=1)[:, :N]
    return full, res


def kernel(x):
    full, _ = _run(x)
    return full


# revision 8
# speedup vs baseline: 1.7518x; 1.0006x over previous
"""Trainium2 Bass kernel (final): sliding-window Pearson correlation attention.

Same math/precision scheme as v3 (bf16 pipeline, f32 ln link, DVE-only scans,
act-table preload), plus software-pipelined emission: each iteration emits the
NEXT iteration's DMA/sq/p12 before the current iteration's dependent tail, so
the in-order engine queues never sit behind a cross-engine wait with runnable
work elsewhere in the program order.

Engine split per (chunk, batch-tile) iteration (model ~8.5us):
  DVE:  5 scans, v, m12, cov, corr
  ACT:  sq, t, s1w, ln, rs (+avgb per chunk)
  Pool: p12, p, r, relu
  PE:   batch-sum matmuls (bf16, M=128 replicated)
"""

import numpy as np

import concourse.bass as bass
import concourse.mybir as mybir
import concourse.tile as tile
from concourse.bass_utils import run_bass_kernel_spmd

WIN = 100
B = 512
CH = 2
T = 32768
N = T - WIN + 1
NCORES = 8
NLOC = 4084
FIN = NLOC + WIN - 1  # 4183
TPAD = (NCORES - 1) * NLOC + FIN
P = 128
NBT = B // P  # 4
NCHUNK = 4
F = NLOC // NCHUNK  # 1021
H = F + WIN  # 1121

f32 = mybir.dt.float32
bf16 = mybir.dt.bfloat16
AOT = mybir.ActivationFunctionType
ALU = mybir.AluOpType
AXL = mybir.AxisListType

NIT = NCHUNK * NBT  # 16 iterations, chunk-major: i = c*NBT + bt


def _kernel_body(tc, out, xs):
    nc = tc.nc
    import contextlib

    ctx = contextlib.ExitStack()
    with ctx:
        const_pool = ctx.enter_context(tc.tile_pool(name="const", bufs=1))
        pool = ctx.enter_context(tc.tile_pool(name="work", bufs=3))
        xpool = ctx.enter_context(tc.tile_pool(name="xin", bufs=4))
        scan_pool = ctx.enter_context(tc.tile_pool(name="scans", bufs=5))
        corr_pool = ctx.enter_context(tc.tile_pool(name="corrp", bufs=6))
        row_pool = ctx.enter_context(tc.tile_pool(name="rows", bufs=3))
        psum_pool = ctx.enter_context(tc.tile_pool(name="psum", bufs=2, space="PSUM"))
        psr_pool = ctx.enter_context(tc.tile_pool(name="psumr", bufs=2, space="PSUM"))

        ones = const_pool.tile([P, P], bf16, tag="ones")
        nc.vector.memset(ones[:], 1.0)
        # identity matrix: keep ones where (free_idx - partition_idx) == 0
        ident = const_pool.tile([P, P], bf16, tag="ident")
        nc.gpsimd.affine_select(
            ident[:], ones[:], pattern=[[1, P]],
            compare_op=ALU.is_equal, fill=0.0, base=0, channel_multiplier=-1,
        )
        plnw = const_pool.tile([P, 1], f32, tag="plnw")
        nc.vector.memset(plnw[:], float(np.log(WIN)))

        SQW = float(np.sqrt(WIN))

        nc.scalar.add_instruction(
            mybir.InstLoadActFuncSet(
                name=nc.get_next_instruction_name(), act_func_set_id=6
            )
        )

        # pipeline state, indexed by iteration
        S = [dict() for _ in range(NIT)]
        prev_scans = [None] * NBT  # per-bt (slin, squad, s12t) for chunk chaining
        corrs = [None] * NBT
        ps_tiles = {}
        avgb_tiles = {}
        corr_final = {}

        def stage_load(i):
            c, bt = divmod(i, NBT)
            b0 = bt * P
            x12 = xpool.tile([P, CH, H], f32, tag="x12")
            g0 = 0 if c == 0 else c * F - 1
            nc.sync.dma_start(out=x12[:], in_=xs[b0 : b0 + P, :, g0 : g0 + H])
            S[i]["x12"] = x12

        def stage_pre(i):
            # sq (ACT) + p12 (Pool) from x12
            x12 = S[i]["x12"]
            sq = pool.tile([P, CH, H], bf16, tag="sq")
            nc.scalar.activation(sq[:], x12[:], AOT.Square, scale=SQW)
            p12 = pool.tile([P, H], bf16, tag="p12")
            nc.gpsimd.tensor_mul(p12[:], x12[:, 0, :], x12[:, 1, :])
            S[i]["sq"] = sq
            S[i]["p12"] = p12

        def stage_scans(i):
            c, bt = divmod(i, NBT)
            x12, sq, p12 = S[i]["x12"], S[i]["sq"], S[i]["p12"]
            slin = scan_pool.tile([P, CH, F], bf16, tag="slin")
            squad = scan_pool.tile([P, CH, F], bf16, tag="squad")
            s12t = scan_pool.tile([P, F], bf16, tag="s12t")

            def wsum(dst2d, src2d, pv):
                if c == 0:
                    with nc.allow_low_precision(reason="f32 accum, bf16 store"):
                        nc.vector.tensor_reduce(
                            out=dst2d[:, 0:1], in_=src2d[:, 0:WIN],
                            op=ALU.add, axis=AXL.X,
                        )
                    nc.vector.tensor_tensor_scan(
                        out=dst2d[:, 1:F],
                        data0=src2d[:, WIN : WIN + F - 1],
                        data1=src2d[:, 0 : F - 1],
                        initial=dst2d[:, 0:1],
                        op0=ALU.add, op1=ALU.subtract,
                    )
                else:
                    nc.vector.tensor_tensor_scan(
                        out=dst2d[:, 0:F],
                        data0=src2d[:, WIN : WIN + F],
                        data1=src2d[:, 0:F],
                        initial=pv,
                        op0=ALU.add, op1=ALU.subtract,
                    )

            pv = prev_scans[bt]
            wsum(slin[:, 0, :], x12[:, 0, :], pv and pv[0][:, 0, F - 1 : F])
            wsum(slin[:, 1, :], x12[:, 1, :], pv and pv[0][:, 1, F - 1 : F])
            wsum(squad[:, 0, :], sq[:, 0, :], pv and pv[1][:, 0, F - 1 : F])
            wsum(squad[:, 1, :], sq[:, 1, :], pv and pv[1][:, 1, F - 1 : F])
            wsum(s12t[:], p12[:], pv and pv[2][:, F - 1 : F])
            prev_scans[bt] = (slin, squad, s12t)
            S[i]["slin"], S[i]["squad"], S[i]["s12t"] = slin, squad, s12t

        def stage_mid(i):
            slin, squad, s12t = S[i]["slin"], S[i]["squad"], S[i]["s12t"]
            t = pool.tile([P, CH, F], bf16, tag="t")
            nc.scalar.activation(t[:], slin[:], AOT.Square)
            s1w = pool.tile([P, F], bf16, tag="s1w")
            nc.scalar.mul(s1w[:], slin[:, 0, :], 1.0 / WIN)
            v = pool.tile([P, CH, F], bf16, tag="v")
            nc.vector.tensor_sub(v[:], squad[:], t[:])
            m12 = pool.tile([P, F], bf16, tag="m12")
            nc.gpsimd.tensor_mul(m12[:], s1w[:], slin[:, 1, :])
            cov = pool.tile([P, F], bf16, tag="cov")
            nc.gpsimd.tensor_sub(cov[:], s12t[:], m12[:])
            p = pool.tile([P, F], bf16, tag="p")
            nc.vector.tensor_mul(p[:], v[:, 0, :], v[:, 1, :])
            S[i]["cov"], S[i]["p"] = cov, p

        def stage_tail(i):
            c, bt = divmod(i, NBT)
            cov, p = S[i]["cov"], S[i]["p"]
            lnp = pool.tile([P, F], f32, tag="lnp")
            nc.scalar.activation(lnp[:], p[:], AOT.Ln)
            rs = pool.tile([P, F], bf16, tag="rs")
            nc.scalar.activation(rs[:], lnp[:], AOT.Exp, scale=-0.5, bias=plnw[:])
            corr = corr_pool.tile([P, F], bf16, tag="corr")
            nc.vector.tensor_mul(corr[:], cov[:], rs[:])
            corrs[bt] = corr
            corr_final[(c, bt)] = corr
            if bt == 0:
                ps_tiles[c] = psum_pool.tile(
                    [P, 1024], f32, tag="ps", name=f"ps{c}"
                )
            ps = ps_tiles[c]
            nc.tensor.matmul(
                ps[:, 0:512], ones[:], corr[:, 0:512],
                start=(bt == 0), stop=(bt == NBT - 1),
            )
            nc.tensor.matmul(
                ps[:, 512:F], ones[:], corr[:, 512:F],
                start=(bt == 0), stop=(bt == NBT - 1),
            )

        def stage_finalize_avgb(c):
            ps = ps_tiles.pop(c)
            avgb = row_pool.tile([P, F], bf16, tag="avgb", name=f"avgb{c}")
            nc.scalar.mul(avgb[:, 0:512], ps[:, 0:512], -1.0 / B)
            nc.scalar.mul(avgb[:, 512:F], ps[:, 512:F], -1.0 / B)
            avgb_tiles[c] = avgb

        def stage_finalize_bt(c, bt, last):
            c0 = c * F
            avgb = avgb_tiles[c]
            b0 = bt * P
            corr = corr_final[(c, bt)]
            # r = corr - mean on the PE: identity-matmul accumulate
            psr = psr_pool.tile([P, 1024], f32, tag="psr", name=f"psr{c}_{bt}")
            for lo, hi in ((0, 512), (512, F)):
                nc.tensor.matmul(
                    psr[:, lo:hi], ident[:], corr[:, lo:hi],
                    start=True, stop=False,
                )
                nc.tensor.matmul(
                    psr[:, lo:hi], ident[:], avgb[:, lo:hi],
                    start=False, stop=True,
                )
            r = pool.tile([P, F], bf16, tag="r")
            if last and bt % 2 == 0:
                nc.vector.tensor_scalar_max(r[:], psr[:, 0:F], 0.0)
            else:
                nc.scalar.activation(r[:], psr[:, 0:F], AOT.Relu)
            nc.sync.dma_start(out=out[b0 : b0 + P, c0 : c0 + F], in_=r[:])

        # software-pipelined emission; tail is delayed one iteration so the
        # ACT queue orders t/s1w(i) ahead of ln/rs(i-1) (avoids head-of-line
        # blocking on the Pool p(i-1) dependency)
        fin_q = []  # (c, bt, last) pending spread finalizes

        def pump_finalize(n):
            for _ in range(min(n, len(fin_q))):
                stage_finalize_bt(*fin_q.pop(0))

        stage_load(0)
        stage_pre(0)
        stage_load(1)
        for i in range(NIT):
            if i + 1 < NIT:
                stage_pre(i + 1)
            if i + 2 < NIT:
                stage_load(i + 2)
            stage_scans(i)
            stage_mid(i)
            pump_finalize(1)
            if i >= 1:
                stage_tail(i - 1)
                c, bt = divmod(i - 1, NBT)
                if bt == NBT - 1:
                    stage_finalize_avgb(c)
                    fin_q.extend((c, b, c == NCHUNK - 1) for b in range(NBT))
        stage_tail(NIT - 1)
        stage_finalize_avgb(NCHUNK - 1)
        fin_q.extend((NCHUNK - 1, b, True) for b in range(NBT))
        pump_finalize(len(fin_q))


def build_nc():
    from concourse import bacc

    nc = bacc.Bacc("TRN2", target_bir_lowering=False, debug=False, num_devices=NCORES)
    xs = nc.dram_tensor("xs", [B, CH, FIN], f32, kind="ExternalInput").ap()
    out = nc.dram_tensor("out", [B, NLOC], bf16, kind="ExternalOutput").ap()
    with tile.TileContext(nc) as tc:
        _kernel_body(tc, out, xs)
    nc.compile()
    return nc


_NC = None


def _get_nc():
    global _NC
    if _NC is None:
        _NC = build_nc()
    return _NC


def make_in_maps(x):
    x = np.asarray(x, dtype=np.float32)
    xpad = np.zeros((B, CH, TPAD), dtype=np.float32)
    xpad[:, :, :T] = x
    return [
        {"xs": np.ascontiguousarray(xpad[:, :, c * NLOC : c * NLOC + FIN])}
        for c in range(NCORES)
    ]


def _run(x, **kwargs):
    nc = _get_nc()
    res = run_bass_kernel_spmd(nc, make_in_maps(x), core_ids=list(range(NCORES)), **kwargs)
    outs = [np.asarray(res.results[c]["out"]).astype(np.float32) for c in range(NCORES)]
    full = np.concatenate(outs, axis=1)[:, :N]
    return full, res


def kernel(x):
    full, _ = _run(x)
    return full


# revision 9
# speedup vs baseline: 1.7655x; 1.0078x over previous
"""Trainium2 Bass kernel (final): sliding-window Pearson correlation attention.

Same math/precision scheme as v3 (bf16 pipeline, f32 ln link, DVE-only scans,
act-table preload), plus software-pipelined emission: each iteration emits the
NEXT iteration's DMA/sq/p12 before the current iteration's dependent tail, so
the in-order engine queues never sit behind a cross-engine wait with runnable
work elsewhere in the program order.

Engine split per (chunk, batch-tile) iteration (model ~8.5us):
  DVE:  5 scans, v, m12, cov, corr
  ACT:  sq, t, s1w, ln, rs (+avgb per chunk)
  Pool: p12, p, r, relu
  PE:   batch-sum matmuls (bf16, M=128 replicated)
"""

import numpy as np

import concourse.bass as bass
import concourse.mybir as mybir
import concourse.tile as tile
from concourse.bass_utils import run_bass_kernel_spmd

WIN = 100
B = 512
CH = 2
T = 32768
N = T - WIN + 1
NCORES = 8
NLOC = 4084
FIN = NLOC + WIN - 1  # 4183
TPAD = (NCORES - 1) * NLOC + FIN
P = 128
NBT = B // P  # 4
NCHUNK = 4
F = NLOC // NCHUNK  # 1021
H = F + WIN  # 1121

f32 = mybir.dt.float32
bf16 = mybir.dt.bfloat16
AOT = mybir.ActivationFunctionType
ALU = mybir.AluOpType
AXL = mybir.AxisListType

NIT = NCHUNK * NBT  # 16 iterations, chunk-major: i = c*NBT + bt


def _kernel_body(tc, out, xs):
    nc = tc.nc
    import contextlib

    ctx = contextlib.ExitStack()
    with ctx:
        const_pool = ctx.enter_context(tc.tile_pool(name="const", bufs=1))
        pool = ctx.enter_context(tc.tile_pool(name="work", bufs=3))
        xpool = ctx.enter_context(tc.tile_pool(name="xin", bufs=4))
        scan_pool = ctx.enter_context(tc.tile_pool(name="scans", bufs=5))
        corr_pool = ctx.enter_context(tc.tile_pool(name="corrp", bufs=6))
        row_pool = ctx.enter_context(tc.tile_pool(name="rows", bufs=3))
        psum_pool = ctx.enter_context(tc.tile_pool(name="psum", bufs=2, space="PSUM"))
        psr_pool = ctx.enter_context(tc.tile_pool(name="psumr", bufs=2, space="PSUM"))

        ones = const_pool.tile([P, P], bf16, tag="ones")
        nc.vector.memset(ones[:], 1.0)
        # identity matrix: keep ones where (free_idx - partition_idx) == 0
        ident = const_pool.tile([P, P], bf16, tag="ident")
        nc.gpsimd.affine_select(
            ident[:], ones[:], pattern=[[1, P]],
            compare_op=ALU.is_equal, fill=0.0, base=0, channel_multiplier=-1,
        )
        plnw = const_pool.tile([P, 1], f32, tag="plnw")
        nc.vector.memset(plnw[:], float(np.log(WIN)))

        SQW = float(np.sqrt(WIN))

        nc.scalar.add_instruction(
            mybir.InstLoadActFuncSet(
                name=nc.get_next_instruction_name(), act_func_set_id=6
            )
        )

        # pipeline state, indexed by iteration
        S = [dict() for _ in range(NIT)]
        prev_scans = [None] * NBT  # per-bt (slin, squad, s12t) for chunk chaining
        corrs = [None] * NBT
        ps_tiles = {}
        avgb_tiles = {}
        corr_final = {}

        def stage_load(i):
            c, bt = divmod(i, NBT)
            b0 = bt * P
            x12 = xpool.tile([P, CH, H], f32, tag="x12")
            g0 = 0 if c == 0 else c * F - 1
            if i == 0:
                # cold start: split the first tile's load so compute begins
                # after half the transfer
                nc.sync.dma_start(out=x12[:, :, 0:561], in_=xs[b0 : b0 + P, :, 0:561])
                nc.sync.dma_start(out=x12[:, :, 561:H], in_=xs[b0 : b0 + P, :, 561:H])
            else:
                nc.sync.dma_start(out=x12[:], in_=xs[b0 : b0 + P, :, g0 : g0 + H])
            S[i]["x12"] = x12

        def stage_pre(i):
            # sq (ACT) + p12 (Pool) from x12
            x12 = S[i]["x12"]
            sq = pool.tile([P, CH, H], bf16, tag="sq")
            p12 = pool.tile([P, H], bf16, tag="p12")
            if i == 0:
                nc.scalar.activation(sq[:, :, 0:561], x12[:, :, 0:561], AOT.Square, scale=SQW)
                nc.scalar.activation(sq[:, :, 561:H], x12[:, :, 561:H], AOT.Square, scale=SQW)
                nc.gpsimd.tensor_mul(p12[:, 0:561], x12[:, 0, 0:561], x12[:, 1, 0:561])
                nc.gpsimd.tensor_mul(p12[:, 561:H], x12[:, 0, 561:H], x12[:, 1, 561:H])
            else:
                nc.scalar.activation(sq[:], x12[:], AOT.Square, scale=SQW)
                nc.gpsimd.tensor_mul(p12[:], x12[:, 0, :], x12[:, 1, :])
            S[i]["sq"] = sq
            S[i]["p12"] = p12

        def stage_scans(i):
            c, bt = divmod(i, NBT)
            x12, sq, p12 = S[i]["x12"], S[i]["sq"], S[i]["p12"]
            slin = scan_pool.tile([P, CH, F], bf16, tag="slin")
            squad = scan_pool.tile([P, CH, F], bf16, tag="squad")
            s12t = scan_pool.tile([P, F], bf16, tag="s12t")

            def wsum(dst2d, src2d, pv):
                if c == 0:
                    with nc.allow_low_precision(reason="f32 accum, bf16 store"):
                        nc.vector.tensor_reduce(
                            out=dst2d[:, 0:1], in_=src2d[:, 0:WIN],
                            op=ALU.add, axis=AXL.X,
                        )
                    if i == 0:
                        # cold start: scan in two pieces so the first piece
                        # only needs the first half of the input tile
                        nc.vector.tensor_tensor_scan(
                            out=dst2d[:, 1:461],
                            data0=src2d[:, WIN : WIN + 460],
                            data1=src2d[:, 0:460],
                            initial=dst2d[:, 0:1],
                            op0=ALU.add, op1=ALU.subtract,
                        )
                        nc.vector.tensor_tensor_scan(
                            out=dst2d[:, 461:F],
                            data0=src2d[:, WIN + 460 : WIN + F - 1],
                            data1=src2d[:, 460 : F - 1],
                            initial=dst2d[:, 460:461],
                            op0=ALU.add, op1=ALU.subtract,
                        )
                    else:
                        nc.vector.tensor_tensor_scan(
                            out=dst2d[:, 1:F],
                            data0=src2d[:, WIN : WIN + F - 1],
                            data1=src2d[:, 0 : F - 1],
                            initial=dst2d[:, 0:1],
                            op0=ALU.add, op1=ALU.subtract,
                        )
                else:
                    nc.vector.tensor_tensor_scan(
                        out=dst2d[:, 0:F],
                        data0=src2d[:, WIN : WIN + F],
                        data1=src2d[:, 0:F],
                        initial=pv,
                        op0=ALU.add, op1=ALU.subtract,
                    )

            pv = prev_scans[bt]
            wsum(slin[:, 0, :], x12[:, 0, :], pv and pv[0][:, 0, F - 1 : F])
            wsum(slin[:, 1, :], x12[:, 1, :], pv and pv[0][:, 1, F - 1 : F])
            wsum(squad[:, 0, :], sq[:, 0, :], pv and pv[1][:, 0, F - 1 : F])
            wsum(squad[:, 1, :], sq[:, 1, :], pv and pv[1][:, 1, F - 1 : F])
            wsum(s12t[:], p12[:], pv and pv[2][:, F - 1 : F])
            prev_scans[bt] = (slin, squad, s12t)
            S[i]["slin"], S[i]["squad"], S[i]["s12t"] = slin, squad, s12t

        def stage_mid(i):
            slin, squad, s12t = S[i]["slin"], S[i]["squad"], S[i]["s12t"]
            t = pool.tile([P, CH, F], bf16, tag="t")
            nc.scalar.activation(t[:], slin[:], AOT.Square)
            s1w = pool.tile([P, F], bf16, tag="s1w")
            nc.scalar.mul(s1w[:], slin[:, 0, :], 1.0 / WIN)
            v = pool.tile([P, CH, F], bf16, tag="v")
            nc.vector.tensor_sub(v[:], squad[:], t[:])
            m12 = pool.tile([P, F], bf16, tag="m12")
            nc.gpsimd.tensor_mul(m12[:], s1w[:], slin[:, 1, :])
            cov = pool.tile([P, F], bf16, tag="cov")
            nc.gpsimd.tensor_sub(cov[:], s12t[:], m12[:])
            p = pool.tile([P, F], bf16, tag="p")
            nc.vector.tensor_mul(p[:], v[:, 0, :], v[:, 1, :])
            S[i]["cov"], S[i]["p"] = cov, p

        def stage_tail(i):
            c, bt = divmod(i, NBT)
            cov, p = S[i]["cov"], S[i]["p"]
            lnp = pool.tile([P, F], f32, tag="lnp")
            nc.scalar.activation(lnp[:], p[:], AOT.Ln)
            rs = pool.tile([P, F], bf16, tag="rs")
            nc.scalar.activation(rs[:], lnp[:], AOT.Exp, scale=-0.5, bias=plnw[:])
            corr = corr_pool.tile([P, F], bf16, tag="corr")
            nc.vector.tensor_mul(corr[:], cov[:], rs[:])
            corrs[bt] = corr
            corr_final[(c, bt)] = corr
            if bt == 0:
                ps_tiles[c] = psum_pool.tile(
                    [P, 1024], f32, tag="ps", name=f"ps{c}"
                )
            ps = ps_tiles[c]
            nc.tensor.matmul(
                ps[:, 0:512], ones[:], corr[:, 0:512],
                start=(bt == 0), stop=(bt == NBT - 1),
            )
            nc.tensor.matmul(
                ps[:, 512:F], ones[:], corr[:, 512:F],
                start=(bt == 0), stop=(bt == NBT - 1),
            )

        def stage_finalize_avgb(c):
            ps = ps_tiles.pop(c)
            avgb = row_pool.tile([P, F], bf16, tag="avgb", name=f"avgb{c}")
            nc.scalar.mul(avgb[:, 0:512], ps[:, 0:512], -1.0 / B)
            nc.scalar.mul(avgb[:, 512:F], ps[:, 512:F], -1.0 / B)
            avgb_tiles[c] = avgb

        def stage_finalize_bt(c, bt, last):
            c0 = c * F
            avgb = avgb_tiles[c]
            b0 = bt * P
            corr = corr_final[(c, bt)]
            # r = corr - mean on the PE: identity-matmul accumulate
            psr = psr_pool.tile([P, 1024], f32, tag="psr", name=f"psr{c}_{bt}")
            for lo, hi in ((0, 512), (512, F)):
                nc.tensor.matmul(
                    psr[:, lo:hi], ident[:], corr[:, lo:hi],
                    start=True, stop=False,
                )
                nc.tensor.matmul(
                    psr[:, lo:hi], ident[:], avgb[:, lo:hi],
                    start=False, stop=True,
                )
            r = pool.tile([P, F], bf16, tag="r")
            if last and bt % 2 == 0:
                nc.vector.tensor_scalar_max(r[:], psr[:, 0:F], 0.0)
            else:
                nc.scalar.activation(r[:], psr[:, 0:F], AOT.Relu)
            nc.sync.dma_start(out=out[b0 : b0 + P, c0 : c0 + F], in_=r[:])

        # software-pipelined emission; tail is delayed one iteration so the
        # ACT queue orders t/s1w(i) ahead of ln/rs(i-1) (avoids head-of-line
        # blocking on the Pool p(i-1) dependency)
        fin_q = []  # (c, bt, last) pending spread finalizes

        def pump_finalize(n):
            for _ in range(min(n, len(fin_q))):
                stage_finalize_bt(*fin_q.pop(0))

        stage_load(0)
        stage_pre(0)
        stage_load(1)
        for i in range(NIT):
            if i + 1 < NIT:
                stage_pre(i + 1)
            if i + 2 < NIT:
                stage_load(i + 2)
            stage_scans(i)
            stage_mid(i)
            pump_finalize(1)
            if i >= 1:
                stage_tail(i - 1)
                c, bt = divmod(i - 1, NBT)
                if bt == NBT - 1:
                    stage_finalize_avgb(c)
                    fin_q.extend((c, b, c == NCHUNK - 1) for b in range(NBT))
        stage_tail(NIT - 1)
        stage_finalize_avgb(NCHUNK - 1)
        fin_q.extend((NCHUNK - 1, b, True) for b in range(NBT))
        pump_finalize(len(fin_q))


def build_nc():
    from concourse import bacc

    nc = bacc.Bacc("TRN2", target_bir_lowering=False, debug=False, num_devices=NCORES)
    xs = nc.dram_tensor("xs", [B, CH, FIN], f32, kind="ExternalInput").ap()
    out = nc.dram_tensor("out", [B, NLOC], bf16, kind="ExternalOutput").ap()
    with tile.TileContext(nc) as tc:
        _kernel_body(tc, out, xs)
    nc.compile()
    return nc


_NC = None


def _get_nc():
    global _NC
    if _NC is None:
        _NC = build_nc()
    return _NC


def make_in_maps(x):
    x = np.asarray(x, dtype=np.float32)
    xpad = np.zeros((B, CH, TPAD), dtype=np.float32)
    xpad[:, :, :T] = x
    return [
        {"xs": np.ascontiguousarray(xpad[:, :, c * NLOC : c * NLOC + FIN])}
        for c in range(NCORES)
    ]


def _run(x, **kwargs):
    nc = _get_nc()
    res = run_bass_kernel_spmd(nc, make_in_maps(x), core_ids=list(range(NCORES)), **kwargs)
    outs = [np.asarray(res.results[c]["out"]).astype(np.float32) for c in range(NCORES)]
    full = np.concatenate(outs, axis=1)[:, :N]
    return full, res


def kernel(x):
    full, _ = _run(x)
    return full


# revision 10
# speedup vs baseline: 1.7696x; 1.0023x over previous
"""Trainium2 Bass kernel (final): sliding-window Pearson correlation attention.

Same math/precision scheme as v3 (bf16 pipeline, f32 ln link, DVE-only scans,
act-table preload), plus software-pipelined emission: each iteration emits the
NEXT iteration's DMA/sq/p12 before the current iteration's dependent tail, so
the in-order engine queues never sit behind a cross-engine wait with runnable
work elsewhere in the program order.

Engine split per (chunk, batch-tile) iteration (model ~8.5us):
  DVE:  5 scans, v, m12, cov, corr
  ACT:  sq, t, s1w, ln, rs (+avgb per chunk)
  Pool: p12, p, r, relu
  PE:   batch-sum matmuls (bf16, M=128 replicated)
"""

import numpy as np

import concourse.bass as bass
import concourse.mybir as mybir
import concourse.tile as tile
from concourse.bass_utils import run_bass_kernel_spmd

WIN = 100
B = 512
CH = 2
T = 32768
N = T - WIN + 1
NCORES = 8
NLOC = 4084
FIN = NLOC + WIN - 1  # 4183
TPAD = (NCORES - 1) * NLOC + FIN
P = 128
NBT = B // P  # 4
NCHUNK = 4
F = NLOC // NCHUNK  # 1021
H = F + WIN  # 1121

f32 = mybir.dt.float32
bf16 = mybir.dt.bfloat16
AOT = mybir.ActivationFunctionType
ALU = mybir.AluOpType
AXL = mybir.AxisListType

NIT = NCHUNK * NBT  # 16 iterations, chunk-major: i = c*NBT + bt


def _kernel_body(tc, out, xs):
    nc = tc.nc
    import contextlib

    ctx = contextlib.ExitStack()
    with ctx:
        const_pool = ctx.enter_context(tc.tile_pool(name="const", bufs=1))
        pool = ctx.enter_context(tc.tile_pool(name="work", bufs=3))
        xpool = ctx.enter_context(tc.tile_pool(name="xin", bufs=4))
        scan_pool = ctx.enter_context(tc.tile_pool(name="scans", bufs=5))
        corr_pool = ctx.enter_context(tc.tile_pool(name="corrp", bufs=6))
        row_pool = ctx.enter_context(tc.tile_pool(name="rows", bufs=3))
        psum_pool = ctx.enter_context(tc.tile_pool(name="psum", bufs=2, space="PSUM"))
        psr_pool = ctx.enter_context(tc.tile_pool(name="psumr", bufs=2, space="PSUM"))

        ones = const_pool.tile([P, P], bf16, tag="ones")
        nc.vector.memset(ones[:], 1.0)
        # identity matrix: keep ones where (free_idx - partition_idx) == 0
        ident = const_pool.tile([P, P], bf16, tag="ident")
        nc.gpsimd.affine_select(
            ident[:], ones[:], pattern=[[1, P]],
            compare_op=ALU.is_equal, fill=0.0, base=0, channel_multiplier=-1,
        )
        plnw = const_pool.tile([P, 1], f32, tag="plnw")
        nc.vector.memset(plnw[:], float(np.log(WIN)))

        SQW = float(np.sqrt(WIN))

        nc.scalar.add_instruction(
            mybir.InstLoadActFuncSet(
                name=nc.get_next_instruction_name(), act_func_set_id=6
            )
        )

        # pipeline state, indexed by iteration
        S = [dict() for _ in range(NIT)]
        prev_scans = [None] * NBT  # per-bt (slin, squad, s12t) for chunk chaining
        corrs = [None] * NBT
        ps_tiles = {}
        avgb_tiles = {}
        corr_final = {}

        def stage_load(i):
            c, bt = divmod(i, NBT)
            b0 = bt * P
            x12 = xpool.tile([P, CH, H], f32, tag="x12")
            g0 = 0 if c == 0 else c * F - 1
            if i == 0:
                # cold start: split the first tile's load so compute begins
                # after half the transfer
                nc.sync.dma_start(out=x12[:, :, 0:281], in_=xs[b0 : b0 + P, :, 0:281])
                nc.sync.dma_start(out=x12[:, :, 281:561], in_=xs[b0 : b0 + P, :, 281:561])
                nc.sync.dma_start(out=x12[:, :, 561:H], in_=xs[b0 : b0 + P, :, 561:H])
            else:
                nc.sync.dma_start(out=x12[:], in_=xs[b0 : b0 + P, :, g0 : g0 + H])
            S[i]["x12"] = x12

        def stage_pre(i):
            # sq (ACT) + p12 (Pool) from x12
            x12 = S[i]["x12"]
            sq = pool.tile([P, CH, H], bf16, tag="sq")
            p12 = pool.tile([P, H], bf16, tag="p12")
            if i == 0:
                nc.scalar.activation(sq[:, :, 0:281], x12[:, :, 0:281], AOT.Square, scale=SQW)
                nc.scalar.activation(sq[:, :, 281:561], x12[:, :, 281:561], AOT.Square, scale=SQW)
                nc.scalar.activation(sq[:, :, 561:H], x12[:, :, 561:H], AOT.Square, scale=SQW)
                nc.gpsimd.tensor_mul(p12[:, 0:281], x12[:, 0, 0:281], x12[:, 1, 0:281])
                nc.gpsimd.tensor_mul(p12[:, 281:561], x12[:, 0, 281:561], x12[:, 1, 281:561])
                nc.gpsimd.tensor_mul(p12[:, 561:H], x12[:, 0, 561:H], x12[:, 1, 561:H])
            else:
                nc.scalar.activation(sq[:], x12[:], AOT.Square, scale=SQW)
                nc.gpsimd.tensor_mul(p12[:], x12[:, 0, :], x12[:, 1, :])
            S[i]["sq"] = sq
            S[i]["p12"] = p12

        def stage_scans(i):
            c, bt = divmod(i, NBT)
            x12, sq, p12 = S[i]["x12"], S[i]["sq"], S[i]["p12"]
            slin = scan_pool.tile([P, CH, F], bf16, tag="slin")
            squad = scan_pool.tile([P, CH, F], bf16, tag="squad")
            s12t = scan_pool.tile([P, F], bf16, tag="s12t")

            def wsum(dst2d, src2d, pv):
                if c == 0:
                    with nc.allow_low_precision(reason="f32 accum, bf16 store"):
                        nc.vector.tensor_reduce(
                            out=dst2d[:, 0:1], in_=src2d[:, 0:WIN],
                            op=ALU.add, axis=AXL.X,
                        )
                    if i == 0:
                        # cold start: scan in two pieces so the first piece
                        # only needs the first half of the input tile
                        nc.vector.tensor_tensor_scan(
                            out=dst2d[:, 1:181],
                            data0=src2d[:, WIN : WIN + 180],
                            data1=src2d[:, 0:180],
                            initial=dst2d[:, 0:1],
                            op0=ALU.add, op1=ALU.subtract,
                        )
                        nc.vector.tensor_tensor_scan(
                            out=dst2d[:, 181:461],
                            data0=src2d[:, WIN + 180 : WIN + 460],
                            data1=src2d[:, 180:460],
                            initial=dst2d[:, 180:181],
                            op0=ALU.add, op1=ALU.subtract,
                        )
                        nc.vector.tensor_tensor_scan(
                            out=dst2d[:, 461:F],
                            data0=src2d[:, WIN + 460 : WIN + F - 1],
                            data1=src2d[:, 460 : F - 1],
                            initial=dst2d[:, 460:461],
                            op0=ALU.add, op1=ALU.subtract,
                        )
                    else:
                        nc.vector.tensor_tensor_scan(
                            out=dst2d[:, 1:F],
                            data0=src2d[:, WIN : WIN + F - 1],
                            data1=src2d[:, 0 : F - 1],
                            initial=dst2d[:, 0:1],
                            op0=ALU.add, op1=ALU.subtract,
                        )
                else:
                    nc.vector.tensor_tensor_scan(
                        out=dst2d[:, 0:F],
                        data0=src2d[:, WIN : WIN + F],
                        data1=src2d[:, 0:F],
                        initial=pv,
                        op0=ALU.add, op1=ALU.subtract,
                    )

            pv = prev_scans[bt]
            wsum(slin[:, 0, :], x12[:, 0, :], pv and pv[0][:, 0, F - 1 : F])
            wsum(slin[:, 1, :], x12[:, 1, :], pv and pv[0][:, 1, F - 1 : F])
            wsum(squad[:, 0, :], sq[:, 0, :], pv and pv[1][:, 0, F - 1 : F])
            wsum(squad[:, 1, :], sq[:, 1, :], pv and pv[1][:, 1, F - 1 : F])
            wsum(s12t[:], p12[:], pv and pv[2][:, F - 1 : F])
            prev_scans[bt] = (slin, squad, s12t)
            S[i]["slin"], S[i]["squad"], S[i]["s12t"] = slin, squad, s12t

        def stage_mid(i):
            slin, squad, s12t = S[i]["slin"], S[i]["squad"], S[i]["s12t"]
            t = pool.tile([P, CH, F], bf16, tag="t")
            nc.scalar.activation(t[:], slin[:], AOT.Square)
            s1w = pool.tile([P, F], bf16, tag="s1w")
            nc.scalar.mul(s1w[:], slin[:, 0, :], 1.0 / WIN)
            v = pool.tile([P, CH, F], bf16, tag="v")
            nc.vector.tensor_sub(v[:], squad[:], t[:])
            m12 = pool.tile([P, F], bf16, tag="m12")
            nc.gpsimd.tensor_mul(m12[:], s1w[:], slin[:, 1, :])
            cov = pool.tile([P, F], bf16, tag="cov")
            nc.gpsimd.tensor_sub(cov[:], s12t[:], m12[:])
            p = pool.tile([P, F], bf16, tag="p")
            nc.vector.tensor_mul(p[:], v[:, 0, :], v[:, 1, :])
            S[i]["cov"], S[i]["p"] = cov, p

        def stage_tail(i):
            c, bt = divmod(i, NBT)
            cov, p = S[i]["cov"], S[i]["p"]
            lnp = pool.tile([P, F], f32, tag="lnp")
            nc.scalar.activation(lnp[:], p[:], AOT.Ln)
            rs = pool.tile([P, F], bf16, tag="rs")
            nc.scalar.activation(rs[:], lnp[:], AOT.Exp, scale=-0.5, bias=plnw[:])
            corr = corr_pool.tile([P, F], bf16, tag="corr")
            nc.vector.tensor_mul(corr[:], cov[:], rs[:])
            corrs[bt] = corr
            corr_final[(c, bt)] = corr
            if bt == 0:
                ps_tiles[c] = psum_pool.tile(
                    [P, 1024], f32, tag="ps", name=f"ps{c}"
                )
            ps = ps_tiles[c]
            nc.tensor.matmul(
                ps[:, 0:512], ones[:], corr[:, 0:512],
                start=(bt == 0), stop=(bt == NBT - 1),
            )
            nc.tensor.matmul(
                ps[:, 512:F], ones[:], corr[:, 512:F],
                start=(bt == 0), stop=(bt == NBT - 1),
            )

        def stage_finalize_avgb(c):
            ps = ps_tiles.pop(c)
            avgb = row_pool.tile([P, F], bf16, tag="avgb", name=f"avgb{c}")
            nc.scalar.mul(avgb[:, 0:512], ps[:, 0:512], -1.0 / B)
            nc.scalar.mul(avgb[:, 512:F], ps[:, 512:F], -1.0 / B)
            avgb_tiles[c] = avgb

        def stage_finalize_bt(c, bt, last):
            c0 = c * F
            avgb = avgb_tiles[c]
            b0 = bt * P
            corr = corr_final[(c, bt)]
            # r = corr - mean on the PE: identity-matmul accumulate
            psr = psr_pool.tile([P, 1024], f32, tag="psr", name=f"psr{c}_{bt}")
            for lo, hi in ((0, 512), (512, F)):
                nc.tensor.matmul(
                    psr[:, lo:hi], ident[:], corr[:, lo:hi],
                    start=True, stop=False,
                )
                nc.tensor.matmul(
                    psr[:, lo:hi], ident[:], avgb[:, lo:hi],
                    start=False, stop=True,
                )
            r = pool.tile([P, F], bf16, tag="r")
            if last and bt % 2 == 0:
                nc.vector.tensor_scalar_max(r[:], psr[:, 0:F], 0.0)
            else:
                nc.scalar.activation(r[:], psr[:, 0:F], AOT.Relu)
            nc.sync.dma_start(out=out[b0 : b0 + P, c0 : c0 + F], in_=r[:])

        # software-pipelined emission; tail is delayed one iteration so the
        # ACT queue orders t/s1w(i) ahead of ln/rs(i-1) (avoids head-of-line
        # blocking on the Pool p(i-1) dependency)
        fin_q = []  # (c, bt, last) pending spread finalizes

        def pump_finalize(n):
            for _ in range(min(n, len(fin_q))):
                stage_finalize_bt(*fin_q.pop(0))

        stage_load(0)
        stage_pre(0)
        stage_load(1)
        for i in range(NIT):
            if i + 1 < NIT:
                stage_pre(i + 1)
            if i + 2 < NIT:
                stage_load(i + 2)
            stage_scans(i)
            stage_mid(i)
            pump_finalize(1)
            if i >= 1:
                stage_tail(i - 1)
                c, bt = divmod(i - 1, NBT)
                if bt == NBT - 1:
                    stage_finalize_avgb(c)
                    fin_q.extend((c, b, c == NCHUNK - 1) for b in range(NBT))
        stage_tail(NIT - 1)
        stage_finalize_avgb(NCHUNK - 1)
        fin_q.extend((NCHUNK - 1, b, True) for b in range(NBT))
        pump_finalize(len(fin_q))


def build_nc():
    from concourse import bacc

    nc = bacc.Bacc("TRN2", target_bir_lowering=False, debug=False, num_devices=NCORES)
    xs = nc.dram_tensor("xs", [B, CH, FIN], f32, kind="ExternalInput").ap()
    out = nc.dram_tensor("out", [B, NLOC], bf16, kind="ExternalOutput").ap()
    with tile.TileContext(nc) as tc:
        _kernel_body(tc, out, xs)
    nc.compile()
    return nc


_NC = None


def _get_nc():
    global _NC
    if _NC is None:
        _NC = build_nc()
    return _NC


def make_in_maps(x):
    x = np.asarray(x, dtype=np.float32)
    xpad = np.zeros((B, CH, TPAD), dtype=np.float32)
    xpad[:, :, :T] = x
    return [
        {"xs": np.ascontiguousarray(xpad[:, :, c * NLOC : c * NLOC + FIN])}
        for c in range(NCORES)
    ]


def _run(x, **kwargs):
    nc = _get_nc()
    res = run_bass_kernel_spmd(nc, make_in_maps(x), core_ids=list(range(NCORES)), **kwargs)
    outs = [np.asarray(res.results[c]["out"]).astype(np.float32) for c in range(NCORES)]
    full = np.concatenate(outs, axis=1)[:, :N]
    return full, res


def kernel(x):
    full, _ = _run(x)
    return full


# revision 11
# speedup vs baseline: 1.8023x; 1.0185x over previous
"""Trainium2 Bass kernel (final): sliding-window Pearson correlation attention.

Same math/precision scheme as v3 (bf16 pipeline, f32 ln link, DVE-only scans,
act-table preload), plus software-pipelined emission: each iteration emits the
NEXT iteration's DMA/sq/p12 before the current iteration's dependent tail, so
the in-order engine queues never sit behind a cross-engine wait with runnable
work elsewhere in the program order.

Engine split per (chunk, batch-tile) iteration (model ~8.5us):
  DVE:  5 scans, v, m12, cov, corr
  ACT:  sq, t, s1w, ln, rs (+avgb per chunk)
  Pool: p12, p, r, relu
  PE:   batch-sum matmuls (bf16, M=128 replicated)
"""

import numpy as np

import concourse.bass as bass
import concourse.mybir as mybir
import concourse.tile as tile
from concourse.bass_utils import run_bass_kernel_spmd

WIN = 100
B = 512
CH = 2
T = 32768
N = T - WIN + 1
NCORES = 8
NLOC = 4084
FIN = NLOC + WIN - 1  # 4183
TPAD = (NCORES - 1) * NLOC + FIN
P = 128
NBT = B // P  # 4
NCHUNK = 4
F = NLOC // NCHUNK  # 1021
H = F + WIN  # 1121

f32 = mybir.dt.float32
bf16 = mybir.dt.bfloat16
AOT = mybir.ActivationFunctionType
ALU = mybir.AluOpType
AXL = mybir.AxisListType

NIT = NCHUNK * NBT  # 16 iterations, chunk-major: i = c*NBT + bt


def _kernel_body(tc, out, xs):
    nc = tc.nc
    import contextlib

    ctx = contextlib.ExitStack()
    with ctx:
        const_pool = ctx.enter_context(tc.tile_pool(name="const", bufs=1))
        pool = ctx.enter_context(tc.tile_pool(name="work", bufs=3))
        xpool = ctx.enter_context(tc.tile_pool(name="xin", bufs=4))
        scan_pool = ctx.enter_context(tc.tile_pool(name="scans", bufs=5))
        corr_pool = ctx.enter_context(tc.tile_pool(name="corrp", bufs=6))
        row_pool = ctx.enter_context(tc.tile_pool(name="rows", bufs=3))
        psum_pool = ctx.enter_context(tc.tile_pool(name="psum", bufs=2, space="PSUM"))
        psr_pool = ctx.enter_context(tc.tile_pool(name="psumr", bufs=2, space="PSUM"))

        ones = const_pool.tile([P, P], bf16, tag="ones")
        nc.vector.memset(ones[:], 1.0)
        # identity matrix: keep ones where (free_idx - partition_idx) == 0
        ident = const_pool.tile([P, P], bf16, tag="ident")
        nc.gpsimd.affine_select(
            ident[:], ones[:], pattern=[[1, P]],
            compare_op=ALU.is_equal, fill=0.0, base=0, channel_multiplier=-1,
        )


        SQW = float(np.sqrt(WIN))

        nc.scalar.add_instruction(
            mybir.InstLoadActFuncSet(
                name=nc.get_next_instruction_name(), act_func_set_id=15
            )
        )

        # pipeline state, indexed by iteration
        S = [dict() for _ in range(NIT)]
        prev_scans = [None] * NBT  # per-bt (slin, squad, s12t) for chunk chaining
        corrs = [None] * NBT
        ps_tiles = {}
        avgb_tiles = {}
        corr_final = {}

        def stage_load(i):
            c, bt = divmod(i, NBT)
            b0 = bt * P
            x12 = xpool.tile([P, CH, H], f32, tag="x12")
            g0 = 0 if c == 0 else c * F - 1
            if i == 0:
                # cold start: split the first tile's load so compute begins
                # after half the transfer
                nc.sync.dma_start(out=x12[:, :, 0:281], in_=xs[b0 : b0 + P, :, 0:281])
                nc.sync.dma_start(out=x12[:, :, 281:561], in_=xs[b0 : b0 + P, :, 281:561])
                nc.sync.dma_start(out=x12[:, :, 561:H], in_=xs[b0 : b0 + P, :, 561:H])
            else:
                nc.sync.dma_start(out=x12[:], in_=xs[b0 : b0 + P, :, g0 : g0 + H])
            S[i]["x12"] = x12

        def stage_pre(i):
            # sq (ACT) + p12 (Pool) from x12
            x12 = S[i]["x12"]
            sq = pool.tile([P, CH, H], bf16, tag="sq")
            p12 = pool.tile([P, H], bf16, tag="p12")
            if i == 0:
                nc.scalar.activation(sq[:, :, 0:281], x12[:, :, 0:281], AOT.Square, scale=SQW)
                nc.scalar.activation(sq[:, :, 281:561], x12[:, :, 281:561], AOT.Square, scale=SQW)
                nc.scalar.activation(sq[:, :, 561:H], x12[:, :, 561:H], AOT.Square, scale=SQW)
                nc.gpsimd.tensor_mul(p12[:, 0:281], x12[:, 0, 0:281], x12[:, 1, 0:281])
                nc.gpsimd.tensor_mul(p12[:, 281:561], x12[:, 0, 281:561], x12[:, 1, 281:561])
                nc.gpsimd.tensor_mul(p12[:, 561:H], x12[:, 0, 561:H], x12[:, 1, 561:H])
            else:
                nc.scalar.activation(sq[:], x12[:], AOT.Square, scale=SQW)
                nc.gpsimd.tensor_mul(p12[:], x12[:, 0, :], x12[:, 1, :])
            S[i]["sq"] = sq
            S[i]["p12"] = p12

        def stage_scans(i):
            c, bt = divmod(i, NBT)
            x12, sq, p12 = S[i]["x12"], S[i]["sq"], S[i]["p12"]
            slin = scan_pool.tile([P, CH, F], bf16, tag="slin")
            squad = scan_pool.tile([P, CH, F], bf16, tag="squad")
            s12t = scan_pool.tile([P, F], bf16, tag="s12t")

            def wsum(dst2d, src2d, pv):
                if c == 0:
                    with nc.allow_low_precision(reason="f32 accum, bf16 store"):
                        nc.vector.tensor_reduce(
                            out=dst2d[:, 0:1], in_=src2d[:, 0:WIN],
                            op=ALU.add, axis=AXL.X,
                        )
                    if i == 0:
                        # cold start: scan in two pieces so the first piece
                        # only needs the first half of the input tile
                        nc.vector.tensor_tensor_scan(
                            out=dst2d[:, 1:181],
                            data0=src2d[:, WIN : WIN + 180],
                            data1=src2d[:, 0:180],
                            initial=dst2d[:, 0:1],
                            op0=ALU.add, op1=ALU.subtract,
                        )
                        nc.vector.tensor_tensor_scan(
                            out=dst2d[:, 181:461],
                            data0=src2d[:, WIN + 180 : WIN + 460],
                            data1=src2d[:, 180:460],
                            initial=dst2d[:, 180:181],
                            op0=ALU.add, op1=ALU.subtract,
                        )
                        nc.vector.tensor_tensor_scan(
                            out=dst2d[:, 461:F],
                            data0=src2d[:, WIN + 460 : WIN + F - 1],
                            data1=src2d[:, 460 : F - 1],
                            initial=dst2d[:, 460:461],
                            op0=ALU.add, op1=ALU.subtract,
                        )
                    else:
                        nc.vector.tensor_tensor_scan(
                            out=dst2d[:, 1:F],
                            data0=src2d[:, WIN : WIN + F - 1],
                            data1=src2d[:, 0 : F - 1],
                            initial=dst2d[:, 0:1],
                            op0=ALU.add, op1=ALU.subtract,
                        )
                else:
                    nc.vector.tensor_tensor_scan(
                        out=dst2d[:, 0:F],
                        data0=src2d[:, WIN : WIN + F],
                        data1=src2d[:, 0:F],
                        initial=pv,
                        op0=ALU.add, op1=ALU.subtract,
                    )

            pv = prev_scans[bt]
            wsum(slin[:, 0, :], x12[:, 0, :], pv and pv[0][:, 0, F - 1 : F])
            wsum(slin[:, 1, :], x12[:, 1, :], pv and pv[0][:, 1, F - 1 : F])
            wsum(squad[:, 0, :], sq[:, 0, :], pv and pv[1][:, 0, F - 1 : F])
            wsum(squad[:, 1, :], sq[:, 1, :], pv and pv[1][:, 1, F - 1 : F])
            wsum(s12t[:], p12[:], pv and pv[2][:, F - 1 : F])
            prev_scans[bt] = (slin, squad, s12t)
            S[i]["slin"], S[i]["squad"], S[i]["s12t"] = slin, squad, s12t

        def stage_mid(i):
            slin, squad, s12t = S[i]["slin"], S[i]["squad"], S[i]["s12t"]
            t = pool.tile([P, CH, F], bf16, tag="t")
            nc.scalar.activation(t[:], slin[:], AOT.Square)
            s1w = pool.tile([P, F], bf16, tag="s1w")
            nc.scalar.mul(s1w[:], slin[:, 0, :], 1.0 / WIN)
            v = pool.tile([P, CH, F], bf16, tag="v")
            nc.vector.tensor_sub(v[:], squad[:], t[:])
            m12 = pool.tile([P, F], bf16, tag="m12")
            nc.gpsimd.tensor_mul(m12[:], s1w[:], slin[:, 1, :])
            cov = pool.tile([P, F], bf16, tag="cov")
            nc.gpsimd.tensor_sub(cov[:], s12t[:], m12[:])
            p = pool.tile([P, F], bf16, tag="p")
            nc.vector.tensor_mul(p[:], v[:, 0, :], v[:, 1, :])
            S[i]["cov"], S[i]["p"] = cov, p

        def stage_tail(i):
            c, bt = divmod(i, NBT)
            cov, p = S[i]["cov"], S[i]["p"]
            # rs = rsqrt(p/w^2) = rsqrt of the unscaled variance product
            # (p is w^2-scaled and > 0, so Abs_reciprocal_sqrt == rsqrt)
            rs = pool.tile([P, F], bf16, tag="rs")
            nc.scalar.activation(
                rs[:], p[:], AOT.Abs_reciprocal_sqrt, scale=1.0 / (WIN * WIN)
            )
            corr = corr_pool.tile([P, F], bf16, tag="corr")
            nc.vector.tensor_mul(corr[:, 0:612], cov[:, 0:612], rs[:, 0:612])
            nc.gpsimd.tensor_mul(corr[:, 612:F], cov[:, 612:F], rs[:, 612:F])
            corrs[bt] = corr
            corr_final[(c, bt)] = corr
            if bt == 0:
                ps_tiles[c] = psum_pool.tile(
                    [P, 1024], f32, tag="ps", name=f"ps{c}"
                )
            ps = ps_tiles[c]
            nc.tensor.matmul(
                ps[:, 0:512], ones[:], corr[:, 0:512],
                start=(bt == 0), stop=(bt == NBT - 1),
            )
            nc.tensor.matmul(
                ps[:, 512:F], ones[:], corr[:, 512:F],
                start=(bt == 0), stop=(bt == NBT - 1),
            )

        def stage_finalize_avgb(c):
            ps = ps_tiles.pop(c)
            avgb = row_pool.tile([P, F], bf16, tag="avgb", name=f"avgb{c}")
            nc.scalar.mul(avgb[:, 0:512], ps[:, 0:512], -1.0 / B)
            nc.scalar.mul(avgb[:, 512:F], ps[:, 512:F], -1.0 / B)
            avgb_tiles[c] = avgb

        def stage_finalize_bt(c, bt, last):
            c0 = c * F
            avgb = avgb_tiles[c]
            b0 = bt * P
            corr = corr_final[(c, bt)]
            # r = corr - mean on the PE: identity-matmul accumulate
            psr = psr_pool.tile([P, 1024], f32, tag="psr", name=f"psr{c}_{bt}")
            for lo, hi in ((0, 512), (512, F)):
                nc.tensor.matmul(
                    psr[:, lo:hi], ident[:], corr[:, lo:hi],
                    start=True, stop=False,
                )
                nc.tensor.matmul(
                    psr[:, lo:hi], ident[:], avgb[:, lo:hi],
                    start=False, stop=True,
                )
            r = pool.tile([P, F], bf16, tag="r")
            if last and bt % 2 == 0:
                nc.vector.tensor_scalar_max(r[:], psr[:, 0:F], 0.0)
            else:
                nc.scalar.activation(r[:], psr[:, 0:F], AOT.Relu)
            nc.sync.dma_start(out=out[b0 : b0 + P, c0 : c0 + F], in_=r[:])

        # software-pipelined emission; tail is delayed one iteration so the
        # ACT queue orders t/s1w(i) ahead of ln/rs(i-1) (avoids head-of-line
        # blocking on the Pool p(i-1) dependency)
        fin_q = []  # (c, bt, last) pending spread finalizes

        def pump_finalize(n):
            for _ in range(min(n, len(fin_q))):
                stage_finalize_bt(*fin_q.pop(0))

        stage_load(0)
        stage_pre(0)
        stage_load(1)
        for i in range(NIT):
            if i + 1 < NIT:
                stage_pre(i + 1)
            if i + 2 < NIT:
                stage_load(i + 2)
            stage_scans(i)
            stage_mid(i)
            pump_finalize(1)
            if i >= 1:
                stage_tail(i - 1)
                c, bt = divmod(i - 1, NBT)
                if bt == NBT - 1:
                    stage_finalize_avgb(c)
                    fin_q.extend((c, b, c == NCHUNK - 1) for b in range(NBT))
        stage_tail(NIT - 1)
        stage_finalize_avgb(NCHUNK - 1)
        fin_q.extend((NCHUNK - 1, b, True) for b in range(NBT))
        pump_finalize(len(fin_q))


def build_nc():
    from concourse import bacc

    nc = bacc.Bacc("TRN2", target_bir_lowering=False, debug=False, num_devices=NCORES)
    xs = nc.dram_tensor("xs", [B, CH, FIN], f32, kind="ExternalInput").ap()
    out = nc.dram_tensor("out", [B, NLOC], bf16, kind="ExternalOutput").ap()
    with tile.TileContext(nc) as tc:
        _kernel_body(tc, out, xs)
    nc.compile()
    return nc


_NC = None


def _get_nc():
    global _NC
    if _NC is None:
        _NC = build_nc()
    return _NC


def make_in_maps(x):
    x = np.asarray(x, dtype=np.float32)
    xpad = np.zeros((B, CH, TPAD), dtype=np.float32)
    xpad[:, :, :T] = x
    return [
        {"xs": np.ascontiguousarray(xpad[:, :, c * NLOC : c * NLOC + FIN])}
        for c in range(NCORES)
    ]


def _run(x, **kwargs):
    nc = _get_nc()
    res = run_bass_kernel_spmd(nc, make_in_maps(x), core_ids=list(range(NCORES)), **kwargs)
    outs = [np.asarray(res.results[c]["out"]).astype(np.float32) for c in range(NCORES)]
    full = np.concatenate(outs, axis=1)[:, :N]
    return full, res


def kernel(x):
    full, _ = _run(x)
    return full


# revision 12
# speedup vs baseline: 1.8152x; 1.0072x over previous
"""Trainium2 Bass kernel (final): sliding-window Pearson correlation attention.

Same math/precision scheme as v3 (bf16 pipeline, f32 ln link, DVE-only scans,
act-table preload), plus software-pipelined emission: each iteration emits the
NEXT iteration's DMA/sq/p12 before the current iteration's dependent tail, so
the in-order engine queues never sit behind a cross-engine wait with runnable
work elsewhere in the program order.

Engine split per (chunk, batch-tile) iteration (model ~8.5us):
  DVE:  5 scans, v, m12, cov, corr
  ACT:  sq, t, s1w, ln, rs (+avgb per chunk)
  Pool: p12, p, r, relu
  PE:   batch-sum matmuls (bf16, M=128 replicated)
"""

import numpy as np

import concourse.bass as bass
import concourse.mybir as mybir
import concourse.tile as tile
from concourse.bass_utils import run_bass_kernel_spmd

WIN = 100
B = 512
CH = 2
T = 32768
N = T - WIN + 1
NCORES = 8
NLOC = 4084
FIN = NLOC + WIN - 1  # 4183
TPAD = (NCORES - 1) * NLOC + FIN
P = 128
NBT = B // P  # 4
NCHUNK = 4
F = NLOC // NCHUNK  # 1021
H = F + WIN  # 1121

f32 = mybir.dt.float32
bf16 = mybir.dt.bfloat16
AOT = mybir.ActivationFunctionType
ALU = mybir.AluOpType
AXL = mybir.AxisListType

NIT = NCHUNK * NBT  # 16 iterations, chunk-major: i = c*NBT + bt


def _kernel_body(tc, out, xs):
    nc = tc.nc
    import contextlib

    ctx = contextlib.ExitStack()
    with ctx:
        const_pool = ctx.enter_context(tc.tile_pool(name="const", bufs=1))
        pool = ctx.enter_context(tc.tile_pool(name="work", bufs=3))
        xpool = ctx.enter_context(tc.tile_pool(name="xin", bufs=4))
        scan_pool = ctx.enter_context(tc.tile_pool(name="scans", bufs=5))
        corr_pool = ctx.enter_context(tc.tile_pool(name="corrp", bufs=6))
        row_pool = ctx.enter_context(tc.tile_pool(name="rows", bufs=3))
        psum_pool = ctx.enter_context(tc.tile_pool(name="psum", bufs=2, space="PSUM"))
        psr_pool = ctx.enter_context(tc.tile_pool(name="psumr", bufs=2, space="PSUM"))

        ones = const_pool.tile([P, P], bf16, tag="ones")
        nc.vector.memset(ones[:], 1.0)
        # identity matrix: keep ones where (free_idx - partition_idx) == 0
        ident = const_pool.tile([P, P], bf16, tag="ident")
        nc.gpsimd.affine_select(
            ident[:], ones[:], pattern=[[1, P]],
            compare_op=ALU.is_equal, fill=0.0, base=0, channel_multiplier=-1,
        )


        SQW = float(np.sqrt(WIN))

        nc.scalar.add_instruction(
            mybir.InstLoadActFuncSet(
                name=nc.get_next_instruction_name(), act_func_set_id=15
            )
        )

        # pipeline state, indexed by iteration
        S = [dict() for _ in range(NIT)]
        prev_scans = [None] * NBT  # per-bt (slin, squad, s12t) for chunk chaining
        corrs = [None] * NBT
        ps_tiles = {}
        avgb_tiles = {}
        corr_final = {}

        def stage_load(i):
            c, bt = divmod(i, NBT)
            b0 = bt * P
            x12 = xpool.tile([P, CH, H], f32, tag="x12")
            g0 = 0 if c == 0 else c * F - 1
            if i == 0:
                # cold start: split the first tile's load so compute begins
                # after half the transfer
                nc.sync.dma_start(out=x12[:, :, 0:281], in_=xs[b0 : b0 + P, :, 0:281])
                nc.sync.dma_start(out=x12[:, :, 281:561], in_=xs[b0 : b0 + P, :, 281:561])
                nc.sync.dma_start(out=x12[:, :, 561:H], in_=xs[b0 : b0 + P, :, 561:H])
            else:
                nc.sync.dma_start(out=x12[:], in_=xs[b0 : b0 + P, :, g0 : g0 + H])
            S[i]["x12"] = x12

        def stage_pre(i):
            # sq (ACT) + p12 (Pool) from x12
            x12 = S[i]["x12"]
            sq = pool.tile([P, CH, H], bf16, tag="sq")
            p12 = pool.tile([P, H], bf16, tag="p12")
            if i == 0:
                nc.scalar.activation(sq[:, :, 0:281], x12[:, :, 0:281], AOT.Square, scale=SQW)
                nc.scalar.activation(sq[:, :, 281:561], x12[:, :, 281:561], AOT.Square, scale=SQW)
                nc.scalar.activation(sq[:, :, 561:H], x12[:, :, 561:H], AOT.Square, scale=SQW)
                nc.gpsimd.tensor_mul(p12[:, 0:281], x12[:, 0, 0:281], x12[:, 1, 0:281])
                nc.gpsimd.tensor_mul(p12[:, 281:561], x12[:, 0, 281:561], x12[:, 1, 281:561])
                nc.gpsimd.tensor_mul(p12[:, 561:H], x12[:, 0, 561:H], x12[:, 1, 561:H])
            else:
                nc.scalar.activation(sq[:], x12[:], AOT.Square, scale=SQW)
                nc.gpsimd.tensor_mul(p12[:], x12[:, 0, :], x12[:, 1, :])
            S[i]["sq"] = sq
            S[i]["p12"] = p12

        def stage_scans(i):
            c, bt = divmod(i, NBT)
            x12, sq, p12 = S[i]["x12"], S[i]["sq"], S[i]["p12"]
            slin = scan_pool.tile([P, CH, F], bf16, tag="slin")
            squad = scan_pool.tile([P, CH, F], bf16, tag="squad")
            s12t = scan_pool.tile([P, F], bf16, tag="s12t")

            def wsum(dst2d, src2d, pv):
                if c == 0:
                    with nc.allow_low_precision(reason="f32 accum, bf16 store"):
                        nc.vector.tensor_reduce(
                            out=dst2d[:, 0:1], in_=src2d[:, 0:WIN],
                            op=ALU.add, axis=AXL.X,
                        )
                    if i == 0:
                        # cold start: scan in two pieces so the first piece
                        # only needs the first half of the input tile
                        nc.vector.tensor_tensor_scan(
                            out=dst2d[:, 1:181],
                            data0=src2d[:, WIN : WIN + 180],
                            data1=src2d[:, 0:180],
                            initial=dst2d[:, 0:1],
                            op0=ALU.add, op1=ALU.subtract,
                        )
                        nc.vector.tensor_tensor_scan(
                            out=dst2d[:, 181:461],
                            data0=src2d[:, WIN + 180 : WIN + 460],
                            data1=src2d[:, 180:460],
                            initial=dst2d[:, 180:181],
                            op0=ALU.add, op1=ALU.subtract,
                        )
                        nc.vector.tensor_tensor_scan(
                            out=dst2d[:, 461:F],
                            data0=src2d[:, WIN + 460 : WIN + F - 1],
                            data1=src2d[:, 460 : F - 1],
                            initial=dst2d[:, 460:461],
                            op0=ALU.add, op1=ALU.subtract,
                        )
                    else:
                        nc.vector.tensor_tensor_scan(
                            out=dst2d[:, 1:F],
                            data0=src2d[:, WIN : WIN + F - 1],
                            data1=src2d[:, 0 : F - 1],
                            initial=dst2d[:, 0:1],
                            op0=ALU.add, op1=ALU.subtract,
                        )
                else:
                    nc.vector.tensor_tensor_scan(
                        out=dst2d[:, 0:F],
                        data0=src2d[:, WIN : WIN + F],
                        data1=src2d[:, 0:F],
                        initial=pv,
                        op0=ALU.add, op1=ALU.subtract,
                    )

            pv = prev_scans[bt]
            wsum(slin[:, 0, :], x12[:, 0, :], pv and pv[0][:, 0, F - 1 : F])
            wsum(slin[:, 1, :], x12[:, 1, :], pv and pv[0][:, 1, F - 1 : F])
            wsum(squad[:, 0, :], sq[:, 0, :], pv and pv[1][:, 0, F - 1 : F])
            wsum(squad[:, 1, :], sq[:, 1, :], pv and pv[1][:, 1, F - 1 : F])
            wsum(s12t[:], p12[:], pv and pv[2][:, F - 1 : F])
            prev_scans[bt] = (slin, squad, s12t)
            S[i]["slin"], S[i]["squad"], S[i]["s12t"] = slin, squad, s12t

        def stage_mid(i):
            slin, squad, s12t = S[i]["slin"], S[i]["squad"], S[i]["s12t"]
            t = pool.tile([P, CH, F], bf16, tag="t")
            nc.scalar.activation(t[:], slin[:], AOT.Square)
            s1w = pool.tile([P, F], bf16, tag="s1w")
            nc.scalar.mul(s1w[:], slin[:, 0, :], 1.0 / WIN)
            v = pool.tile([P, CH, F], bf16, tag="v")
            nc.vector.tensor_sub(v[:], squad[:], t[:])
            m12 = pool.tile([P, F], bf16, tag="m12")
            nc.gpsimd.tensor_mul(m12[:], s1w[:], slin[:, 1, :])
            cov = pool.tile([P, F], bf16, tag="cov")
            nc.gpsimd.tensor_sub(cov[:], s12t[:], m12[:])
            p = pool.tile([P, F], bf16, tag="p")
            nc.vector.tensor_mul(p[:], v[:, 0, :], v[:, 1, :])
            S[i]["cov"], S[i]["p"] = cov, p

        def stage_tail(i):
            c, bt = divmod(i, NBT)
            cov, p = S[i]["cov"], S[i]["p"]
            # rs = rsqrt(p/w^2) = rsqrt of the unscaled variance product
            # (p is w^2-scaled and > 0, so Abs_reciprocal_sqrt == rsqrt)
            rs = pool.tile([P, F], bf16, tag="rs")
            nc.scalar.activation(
                rs[:], p[:], AOT.Abs_reciprocal_sqrt, scale=1.0 / (WIN * WIN)
            )
            corr = corr_pool.tile([P, F], bf16, tag="corr")
            nc.vector.tensor_mul(corr[:, 0:512], cov[:, 0:512], rs[:, 0:512])
            nc.gpsimd.tensor_mul(corr[:, 512:F], cov[:, 512:F], rs[:, 512:F])
            corrs[bt] = corr
            corr_final[(c, bt)] = corr
            if bt == 0:
                ps_tiles[c] = psum_pool.tile(
                    [P, 1024], f32, tag="ps", name=f"ps{c}"
                )
            ps = ps_tiles[c]
            nc.tensor.matmul(
                ps[:, 0:512], ones[:], corr[:, 0:512],
                start=(bt == 0), stop=(bt == NBT - 1),
            )
            nc.tensor.matmul(
                ps[:, 512:F], ones[:], corr[:, 512:F],
                start=(bt == 0), stop=(bt == NBT - 1),
            )

        def stage_finalize_avgb(c):
            ps = ps_tiles.pop(c)
            avgb = row_pool.tile([P, F], bf16, tag="avgb", name=f"avgb{c}")
            nc.scalar.mul(avgb[:, 0:512], ps[:, 0:512], -1.0 / B)
            nc.scalar.mul(avgb[:, 512:F], ps[:, 512:F], -1.0 / B)
            avgb_tiles[c] = avgb

        def stage_finalize_bt(c, bt, last):
            c0 = c * F
            avgb = avgb_tiles[c]
            b0 = bt * P
            corr = corr_final[(c, bt)]
            # r = corr - mean on the PE: identity-matmul accumulate
            psr = psr_pool.tile([P, 1024], f32, tag="psr", name=f"psr{c}_{bt}")
            for lo, hi in ((0, 512), (512, F)):
                nc.tensor.matmul(
                    psr[:, lo:hi], ident[:], corr[:, lo:hi],
                    start=True, stop=False,
                )
                nc.tensor.matmul(
                    psr[:, lo:hi], ident[:], avgb[:, lo:hi],
                    start=False, stop=True,
                )
            r = pool.tile([P, F], bf16, tag="r")
            if last and bt % 2 == 0:
                nc.vector.tensor_scalar_max(r[:], psr[:, 0:F], 0.0)
            else:
                nc.scalar.activation(r[:], psr[:, 0:F], AOT.Relu)
            nc.sync.dma_start(out=out[b0 : b0 + P, c0 : c0 + F], in_=r[:])

        # software-pipelined emission; tail is delayed one iteration so the
        # ACT queue orders t/s1w(i) ahead of ln/rs(i-1) (avoids head-of-line
        # blocking on the Pool p(i-1) dependency)
        fin_q = []  # (c, bt, last) pending spread finalizes

        def pump_finalize(n):
            for _ in range(min(n, len(fin_q))):
                stage_finalize_bt(*fin_q.pop(0))

        stage_load(0)
        stage_pre(0)
        stage_load(1)
        for i in range(NIT):
            if i + 1 < NIT:
                stage_pre(i + 1)
            if i + 2 < NIT:
                stage_load(i + 2)
            stage_scans(i)
            stage_mid(i)
            pump_finalize(1)
            if i >= 1:
                stage_tail(i - 1)
                c, bt = divmod(i - 1, NBT)
                if bt == NBT - 1:
                    stage_finalize_avgb(c)
                    fin_q.extend((c, b, c == NCHUNK - 1) for b in range(NBT))
        stage_tail(NIT - 1)
        stage_finalize_avgb(NCHUNK - 1)
        fin_q.extend((NCHUNK - 1, b, True) for b in range(NBT))
        pump_finalize(len(fin_q))


def build_nc():
    from concourse import bacc

    nc = bacc.Bacc("TRN2", target_bir_lowering=False, debug=False, num_devices=NCORES)
    xs = nc.dram_tensor("xs", [B, CH, FIN], f32, kind="ExternalInput").ap()
    out = nc.dram_tensor("out", [B, NLOC], bf16, kind="ExternalOutput").ap()
    with tile.TileContext(nc) as tc:
        _kernel_body(tc, out, xs)
    nc.compile()
    return nc


_NC = None


def _get_nc():
    global _NC
    if _NC is None:
        _NC = build_nc()
    return _NC


def make_in_maps(x):
    x = np.asarray(x, dtype=np.float32)
    xpad = np.zeros((B, CH, TPAD), dtype=np.float32)
    xpad[:, :, :T] = x
    return [
        {"xs": np.ascontiguousarray(xpad[:, :, c * NLOC : c * NLOC + FIN])}
        for c in range(NCORES)
    ]


def _run(x, **kwargs):
    nc = _get_nc()
    res = run_bass_kernel_spmd(nc, make_in_maps(x), core_ids=list(range(NCORES)), **kwargs)
    outs = [np.asarray(res.results[c]["out"]).astype(np.float32) for c in range(NCORES)]
    full = np.concatenate(outs, axis=1)[:, :N]
    return full, res


def kernel(x):
    full, _ = _run(x)
    return full
